# revision 35
# baseline (speedup 1.0000x reference)
"""E3Conv Trainium2 kernel: 8-core SPMD, dst-partitioned edges.

Strategy: sort edges by dst; core i owns nodes [1250i,1250(i+1)) and all edges
into them (no all-reduce needed). Per core: node-MLP replicated, bf16 gather
tables for Ai/Ai*recip, radial MLP + tensor-product restructured as one
K=512 matmul per edge tile, windowed one-hot matmul scatter-mean.
"""
import sys, os
sys.path.insert(0, "/opt/trn_rl_repo")
import numpy as np

import concourse.bass as bass
import concourse.tile as tile
from concourse import bacc, mybir
from concourse import bass_utils
from concourse.masks import make_identity

P = 128
N_NODES, N_EDGES, N_GRAPHS = 10000, 131072, 64
N_CORES, NPC, N_WIN = 8, 1250, 10
MAX_RADIUS, N_BASIS = 4.0, 10
STEP = MAX_RADIUS / (N_BASIS + 1)
VCENters = np.linspace(0.0, MAX_RADIUS, N_BASIS + 2)[1:-1].astype(np.float32)
F32, BF16, F32R, I16 = (mybir.dt.float32, mybir.dt.bfloat16,
                        mybir.dt.float32r, mybir.dt.int16)
AF = mybir.ActivationFunctionType
ALU = mybir.AluOpType
NCH = 79  # node chunks of 128 (79*128 = 10112 >= 10000)


def _build_consts(fW4):
    s3 = 3.0 ** 0.5
    W4p = np.zeros((512, 224), np.float32)
    offs = {0: 0, 1: 1024, 2: 1536}
    wbase = {0: 0, 1: 16, 2: 24}
    scale_l = {0: 1.0 / 64, 1: s3 / 64, 2: 1.0 / 64}
    for l, mo in enumerate((16, 8, 4)):
        for u in range(8):
            for v in range(8):
                for wl in range(mo):
                    col = offs[l] + (u * 8 + v) * mo + wl
                    w = wbase[l] + wl
                    W4p[np.arange(64) * 8 + v, w * 8 + u] = fW4[:, col] * scale_l[l]
    Sel = np.zeros((4, 64, 128), np.float32)
    for q in range(4):
        for r in range(128):
            Sel[q, 16 * q + r // 8, r] = 1.0
    L2_0 = np.zeros((112, 16), np.float32)
    for r in range(112):
        L2_0[r, r // 8] = 1.0
    L2_x = np.zeros((112, 16), np.float32)
    for r in range(16):
        L2_x[r, 14 + r // 8] = 1.0
    L2_1 = np.zeros((112, 44), np.float32)
    for r in range(112):
        w = 14 + r // 8
        if w < 16:
            pass
        elif w < 24:
            for m in range(3):
                L2_1[r, (w - 16) * 3 + m] = 1.0
        else:
            for m in range(5):
                L2_1[r, 24 + (w - 24) * 5 + m] = 1.0
    Selsh = np.zeros((8, 44), np.float32)
    for w in range(8):
        for m in range(3):
            Selsh[m, w * 3 + m] = 1.0
    for w in range(4):
        for m in range(5):
            Selsh[3 + m, 24 + w * 5 + m] = 1.0
    return W4p, Sel, L2_0, L2_x, L2_1, Selsh


def _host_prep(inputs):
    pos = np.asarray(inputs["pos"], np.float32)
    A = np.asarray(inputs["A"]).astype(np.int64)
    batch = np.asarray(inputs["batch"]).astype(np.int64)
    esrc = np.asarray(inputs["edge_src"]).astype(np.int64)
    edst = np.asarray(inputs["edge_dst"]).astype(np.int64)
    shifts = np.asarray(inputs["edge_shifts"], np.float32)
    cell = np.asarray(inputs["cell"], np.float32)
    counts = np.bincount(edst, minlength=N_NODES).astype(np.float32)
    recipc = 1.0 / np.maximum(counts, 1.0)
    cpn = cell[batch].reshape(N_NODES, 9)
    order = np.argsort(edst, kind="stable")
    wins_all, W_CH = [], 0
    for ci in range(N_CORES):
        lo = ci * NPC
        m = order[(edst[order] >= lo) & (edst[order] < lo + NPC)]
        wins = []
        for w in range(N_WIN):
            wlo = lo + w * P
            whi = min(lo + (w + 1) * P, lo + NPC)
            wm = m[(edst[m] >= wlo) & (edst[m] < whi)]
            wins.append(wm)
            W_CH = max(W_CH, (len(wm) + P - 1) // P)
    # uniform even chunk count per window across all cores
        wins_all.append(wins)
    if W_CH % 2:
        W_CH += 1
    C_TOT = N_WIN * W_CH
    E = C_TOT * P
    onehotA = np.zeros((10, N_NODES), np.float32)
    onehotA[A, np.arange(N_NODES)] = 1.0
    recip_pl = np.ones((P, NCH), np.float32)
    rc = np.concatenate([recipc, np.ones(NCH * P - N_NODES, np.float32)])
    recip_pl[:, :] = rc.reshape(NCH, P).T
    per_core = []
    for ci in range(N_CORES):
        idx = np.zeros(E, np.int64)
        valid = np.zeros(E, bool)
        dstloc = np.full(E, -1.0, np.float32)
        for w in range(N_WIN):
            wm = wins_all[ci][w]
            s = w * W_CH * P
            idx[s:s + len(wm)] = wm
            valid[s:s + len(wm)] = True
            dstloc[s:s + len(wm)] = (edst[wm] - ci * NPC - w * P).astype(np.float32)
        src = np.where(valid, esrc[idx], 0)
        dst = np.where(valid, edst[idx], 0)
        sh = np.where(valid[:, None], shifts[idx], np.float32(1.0))
        geom = np.concatenate([pos[src], pos[dst], sh, cpn[src]], 1)  # [E,18]
        geom_pl = np.ascontiguousarray(
            np.transpose(geom.reshape(C_TOT, P, 18), (1, 2, 0)).reshape(P, 18 * C_TOT))
        dst_pl = np.ascontiguousarray(dstloc.reshape(C_TOT, P).T)

        def wrap(ix):
            wr = ix.astype(np.int16).reshape(-1, 16).T  # [16, E/16]
            return np.ascontiguousarray(np.tile(wr, (8, 1)))
        ohm = (dstloc.reshape(C_TOT, P, 1) ==
               np.arange(P, dtype=np.float32)[None, None, :])
        oh_pl = np.ascontiguousarray(
            np.transpose(ohm, (1, 0, 2)).reshape(P, C_TOT * P))
        per_core.append(dict(geom_pl=geom_pl, dst_pl=dst_pl, oh_pl=oh_pl,
                             idx_src=wrap(src), idx_dst=wrap(dst)))
    return per_core, onehotA, recip_pl, W_CH, C_TOT, E


def _build_bass(W_CH, C_TOT, E, consts):
    W4p, Sel, L2_0, L2_x, L2_1, Selsh = [c.astype(np.float32) for c in consts[:6]]
    TILE_CH = W_CH // 2
    NT = C_TOT // TILE_CH
    ET = TILE_CH * P
    NIW = E // 16
    nc = bacc.Bacc("TRN2", target_bir_lowering=False, debug=False,
                   num_devices=N_CORES)

    def din(name, shape, dt=F32):
        return nc.dram_tensor(name, shape, dt, kind="ExternalInput").ap()

    geom_d = din("geom_pl", [P, 18 * C_TOT])
    dst_d = din("dst_pl", [P, C_TOT])
    ohm_d = din("oh_pl", [P, C_TOT * P], BF16)
    isrc_d = din("idx_src", [P, NIW], I16)
    idst_d = din("idx_dst", [P, NIW], I16)
    ohA_d = din("onehotA", [10, N_NODES], BF16)
    rcp_d = din("recip_pl", [P, NCH])
    TA_d = din("TA", [10, 64], BF16)
    W2_d = din("fit_W2", [64, 32], BF16)
    W3_d = din("fit_W3", [32, 8], BF16)
    fW1_d = din("fc_W1p", [10, 64], BF16)
    fW2_d = din("fc_W2p", [64, 64], BF16)
    fW3_d = din("fc_W3p", [64, 64], BF16)
    W4p_d = din("W4p", [128, 4 * 224], BF16)
    Sel_d = din("Sel", [64, 4 * 128], BF16)
    L20_d = din("L2_0", [112, 16], BF16)
    L2x_d = din("L2_x", [112, 16], BF16)
    L21_d = din("L2_1", [112, 44], BF16)
    Ssh_d = din("Selsh", [8, 44], BF16)
    cv_d = din("cvec", [P, 16])
    out_d = nc.dram_tensor("out", [N_WIN * P, 60], F32, kind="ExternalOutput").ap()

    C = C_TOT
    with tile.TileContext(nc) as tc:
        with tc.tile_pool(name="const", bufs=1) as cp, \
             tc.tile_pool(name="sb", bufs=2) as sp, \
             tc.tile_pool(name="big", bufs=1) as bp, \
             tc.tile_pool(name="ps", bufs=1, space="PSUM") as pp, \
             tc.tile_pool(name="psa", bufs=3, space="PSUM") as pa, \
             tc.tile_pool(name="pswin", bufs=2, space="PSUM") as pw, \
             tc.tile_pool(name="dram", bufs=1, space="DRAM") as dp:
            ident = cp.tile([P, P], F32)
            make_identity(nc, ident[:])
            identb = cp.tile([P, P], BF16)
            nc.vector.tensor_copy(identb[:], ident[:])
            io_i = cp.tile([P, P], mybir.dt.int32)
            nc.gpsimd.iota(io_i[:], pattern=[[1, P]], base=0, channel_multiplier=0)
            io_f = cp.tile([P, P], F32)
            nc.vector.tensor_copy(io_f[:], io_i[:])

            def load_const(dram, shape, dt=F32):
                t = cp.tile(shape, dt, tag=dram.tensor.name)
                nc.sync.dma_start(t[:], dram[:])
                return t
            TA = load_const(TA_d, [10, 64], BF16)
            W2 = load_const(W2_d, [64, 32], BF16)
            W3 = load_const(W3_d, [32, 8], BF16)
            fW1 = load_const(fW1_d, [10, 64], BF16)
            fW2 = load_const(fW2_d, [64, 64], BF16)
            fW3 = load_const(fW3_d, [64, 64], BF16)
            W4pt = load_const(W4p_d, [128, 4 * 224], BF16)
            Selt = load_const(Sel_d, [64, 4 * 128], BF16)
            L20 = load_const(L20_d, [112, 16], BF16)
            L2x = load_const(L2x_d, [112, 16], BF16)
            L21 = load_const(L21_d, [112, 44], BF16)
            Ssh = load_const(Ssh_d, [8, 44], BF16)
            cv = load_const(cv_d, [P, 16])
            rcp = load_const(rcp_d, [P, NCH])
            ohA = bp.tile([10, N_NODES], BF16)
            nc.sync.dma_start(ohA[:], ohA_d[:])
            dstl = bp.tile([P, C], F32)
            nc.sync.dma_start(dstl[:], dst_d[:])
            isrc = bp.tile([P, NIW], I16)
            nc.sync.dma_start(isrc[:], isrc_d[:])
            idst = bp.tile([P, NIW], I16)
            nc.sync.dma_start(idst[:], idst_d[:])


            # ---------------- node MLP + gather tables ----------------
            Tsrc = dp.tile([NCH * P, P], BF16)
            Tdst = dp.tile([NCH * P, P], BF16)
            Ai_sb = bp.tile([8, NCH * P], F32)
            nc.gpsimd.memset(Ai_sb[:], 0.0)
            for j in range(20):
                s = j * 512
                n = min(512, N_NODES - s)
                h1p = pp.tile([64, 512], F32, space="PSUM", tag="pc")
                nc.tensor.matmul(h1p[:, :n], TA[:], ohA[:, s:s + n],
                                 start=True, stop=True)
                h1 = sp.tile([64, 512], BF16, tag="h1n")
                nc.scalar.activation(h1[:, :n], h1p[:, :n], AF.Silu)
                h2p = pp.tile([32, 512], F32, space="PSUM", tag="pc")
                nc.tensor.matmul(h2p[:, :n], W2[:], h1[:, :n],
                                 start=True, stop=True)
                h2 = sp.tile([32, 512], BF16, tag="h2n")
                nc.scalar.activation(h2[:, :n], h2p[:, :n], AF.Silu)
                aip = pa.tile([8, 512], F32, space="PSUM", tag="pa")
                nc.tensor.matmul(aip[:, :n], W3[:], h2[:, :n],
                                 start=True, stop=True)
                nc.vector.tensor_copy(Ai_sb[:, s:s + n], aip[:, :n])
            for c in range(NCH):
                s = c * P
                tp = pp.tile([P, 8], F32, space="PSUM", tag="pc")
                nc.tensor.transpose(tp[:], Ai_sb[:, s:s + P], ident[0:8, 0:8])
                f16a = sp.tile([P, 8], BF16, tag="f16a")
                nc.vector.tensor_copy(f16a[:], tp[:])
                f16b = sp.tile([P, 8], BF16, tag="f16b")
                nc.vector.tensor_tensor(
                    out=f16b[:], in0=tp[:],
                    in1=rcp[:, c:c + 1].to_broadcast([P, 8]), op=ALU.mult)
                repa = sp.tile([P, P], BF16, tag="repa")
                nc.vector.tensor_copy(
                    repa[:].rearrange("p (r v) -> p r v", v=8),
                    f16a[:].unsqueeze(1).to_broadcast([P, 16, 8]))
                repb = sp.tile([P, P], BF16, tag="repb")
                nc.vector.tensor_copy(
                    repb[:].rearrange("p (r v) -> p r v", v=8),
                    f16b[:].unsqueeze(1).to_broadcast([P, 16, 8]))
                nc.sync.dma_start(Tsrc[s:s + P, :], repa[:])
                nc.sync.dma_start(Tdst[s:s + P, :], repb[:])

            # ---------------- geometry (plane layout, whole E) ----------------
            gm = bp.tile([P, 18 * C], F32)
            nc.sync.dma_start(gm[:], geom_d[:])
            g3 = gm[:].rearrange("p (f c) -> p f c", f=18)
            tmp9 = bp.tile([P, 9 * C], F32)
            nc.vector.tensor_tensor(
                out=tmp9[:].rearrange("p (i j c) -> p i j c", i=3, j=3),
                in0=gm[:, 9 * C:18 * C].rearrange("p (i j c) -> p i j c", i=3, j=3),
                in1=gm[:, 6 * C:9 * C].rearrange("p (i c) -> p i c", i=3)
                    .unsqueeze(2).to_broadcast([P, 3, 3, C]),
                op=ALU.mult)
            sv = bp.tile([P, 3 * C], F32)
            nc.vector.tensor_tensor(out=sv[:], in0=tmp9[:, 0:3 * C],
                                    in1=tmp9[:, 3 * C:6 * C], op=ALU.add)
            nc.vector.tensor_tensor(out=sv[:], in0=sv[:],
                                    in1=tmp9[:, 6 * C:9 * C], op=ALU.add)
            ev = bp.tile([P, 3 * C], F32)
            nc.vector.tensor_tensor(out=ev[:], in0=g3[:, 3:6].rearrange("p f c -> p (f c)"),
                                    in1=g3[:, 0:3].rearrange("p f c -> p (f c)"),
                                    op=ALU.subtract)
            nc.vector.tensor_tensor(out=ev[:], in0=ev[:], in1=sv[:], op=ALU.add)
            sq = bp.tile([P, 3 * C], F32)
            nc.vector.tensor_tensor(out=sq[:], in0=ev[:], in1=ev[:], op=ALU.mult)
            ln2 = bp.tile([P, C], F32)
            nc.vector.tensor_tensor(out=ln2[:], in0=sq[:, 0:C], in1=sq[:, C:2 * C],
                                    op=ALU.add)
            nc.vector.tensor_tensor(out=ln2[:], in0=ln2[:], in1=sq[:, 2 * C:3 * C],
                                    op=ALU.add)
            ln = bp.tile([P, C], F32)
            nc.scalar.activation(ln[:], ln2[:], AF.Sqrt)
            rl = bp.tile([P, C], F32)
            nc.vector.reciprocal(rl[:], ln[:])
            u = bp.tile([P, 3 * C], F32)
            nc.vector.tensor_tensor(
                out=u[:].rearrange("p (f c) -> p f c", f=3),
                in0=ev[:].rearrange("p (f c) -> p f c", f=3),
                in1=rl[:].unsqueeze(1).to_broadcast([P, 3, C]), op=ALU.mult)
            usq = bp.tile([P, 3 * C], F32)
            nc.vector.tensor_tensor(out=usq[:], in0=u[:], in1=u[:], op=ALU.mult)
            # feature planes tile: f-major [basis10 | sh1 3 | sh2 5]
            gf = bp.tile([P, 18 * C], F32)
            dt2 = bp.tile([P, 10 * C], F32)
            s5, s15 = 5.0 ** 0.5, 15.0 ** 0.5
            for b in range(N_BASIS):
                nc.scalar.activation(dt2[:, b * C:(b + 1) * C], ln[:], AF.Square,
                                     bias=cv[:, b:b + 1],
                                     scale=cv[:, 10:11])
            nc.scalar.activation(gf[:, 0:10 * C], dt2[:], AF.Exp,
                                 scale=cv[:, 11:12])
            nc.vector.tensor_copy(gf[:, 10 * C:13 * C], u[:])
            t1 = bp.tile([P, C], F32)
            nc.scalar.mul(t1[:], u[:, 2 * C:3 * C], cv[:, 12:13])       # sqrt15*uz
            nc.vector.tensor_tensor(out=gf[:, 13 * C:14 * C], in0=u[:, 0:C],
                                    in1=t1[:], op=ALU.mult)     # m0
            nc.vector.tensor_tensor(out=gf[:, 16 * C:17 * C], in0=u[:, C:2 * C],
                                    in1=t1[:], op=ALU.mult)     # m3
            nc.scalar.mul(t1[:], u[:, 0:C], cv[:, 12:13])               # sqrt15*ux
            nc.vector.tensor_tensor(out=gf[:, 14 * C:15 * C], in0=u[:, C:2 * C],
                                    in1=t1[:], op=ALU.mult)     # m1
            t2 = bp.tile([P, C], F32)
            nc.vector.tensor_tensor(out=t2[:], in0=usq[:, 0:C],
                                    in1=usq[:, 2 * C:3 * C], op=ALU.add)
            nc.scalar.mul(t2[:], t2[:], cv[:, 13:14])
            t3 = bp.tile([P, C], F32)
            nc.scalar.mul(t3[:], usq[:, C:2 * C], cv[:, 14:15])
            nc.vector.tensor_tensor(out=gf[:, 15 * C:16 * C], in0=t3[:], in1=t2[:],
                                    op=ALU.subtract)            # m2
            nc.vector.tensor_tensor(out=t2[:], in0=usq[:, 2 * C:3 * C],
                                    in1=usq[:, 0:C], op=ALU.subtract)
            nc.scalar.mul(gf[:, 17 * C:18 * C], t2[:], cv[:, 15:16])  # m4
            gfv = gf[:].rearrange("p (f c) -> p f c", f=18)

            NSL = [(0, 512), (512, ET)] if ET > 512 else [(0, ET)]
            # ---------------- edge tiles ----------------
            win_sb = None
            for t in range(NT):
                cols = slice(t * ET, (t + 1) * ET)
                wcols = slice(t * (NIW // NT), (t + 1) * (NIW // NT))
                aiS = sp.tile([P, ET], BF16, tag="aiS")
                nc.gpsimd.dma_gather(
                    aiS[:].unsqueeze(1), Tsrc[:, :], isrc[:, wcols], ET, ET, P,
                    transpose=True)
                aiD = sp.tile([P, ET], BF16, tag="aiD")
                nc.gpsimd.dma_gather(
                    aiD[:].unsqueeze(1), Tdst[:, :], idst[:, wcols], ET, ET, P,
                    transpose=True)
                oht = sp.tile([P, ET], BF16, tag="oht")
                nc.sync.dma_start(oht[:], ohm_d[:, t * ET:(t + 1) * ET])
                h1p = pa.tile([64, ET], F32, space="PSUM", tag="pa")
                bts = []
                for cc in range(TILE_CH):
                    cg = t * TILE_CH + cc
                    btp = pp.tile([10, P], F32, space="PSUM", tag="pc")
                    nc.tensor.transpose(btp[:], gfv[:, 0:10, cg], ident[:])
                    bt = sp.tile([10, P], BF16, tag=f"bt{cc}")
                    nc.vector.tensor_copy(bt[:], btp[:])
                    btq = pp.tile([8, P], F32, space="PSUM", tag="pc")
                    nc.tensor.transpose(btq[:], gfv[:, 10:18, cg], ident[:])
                    bsh = sp.tile([8, P], BF16, tag=f"bsh{cc}")
                    nc.vector.tensor_copy(bsh[:], btq[:])
                    bts.append(bsh)
                    csl = slice(cc * P, (cc + 1) * P)
                    nc.tensor.matmul(h1p[:, csl], fW1[:], bt[:],
                                     start=True, stop=True)
                h1 = sp.tile([64, ET], BF16, tag="eh1")
                nc.scalar.activation(h1[:], h1p[:], AF.Silu)
                shp = pa.tile([44, ET], F32, space="PSUM", tag="pa")
                for cc in range(TILE_CH):
                    csl = slice(cc * P, (cc + 1) * P)
                    nc.tensor.matmul(shp[:, csl], Ssh[:], bts[cc][:],
                                     start=True, stop=True)
                shs = sp.tile([44, ET], BF16, tag="shs")
                nc.vector.tensor_copy(shs[:], shp[:])
                h2p = pa.tile([64, ET], F32, space="PSUM", tag="pa")
                for a, b in NSL:
                    nc.tensor.matmul(h2p[:, a:b], fW2[:], h1[:, a:b],
                                     start=True, stop=True)
                h2 = sp.tile([64, ET], BF16, tag="eh2")
                nc.scalar.activation(h2[:], h2p[:], AF.Silu)
                h3p = pa.tile([64, ET], F32, space="PSUM", tag="pa")
                for a, b in NSL:
                    nc.tensor.matmul(h3p[:, a:b], fW3[:], h2[:, a:b],
                                     start=True, stop=True)
                w3b = sp.tile([64, ET], BF16, tag="ew3b")
                nc.scalar.activation(w3b[:], h3p[:], AF.Silu)
                rqs = []
                for q in range(4):
                    wrp = pa.tile([P, ET], F32, space="PSUM", tag="pa")
                    for a, b in NSL:
                        nc.tensor.matmul(wrp[:, a:b],
                                         Selt[:, 128 * q:128 * (q + 1)],
                                         w3b[:, a:b], start=True, stop=True)
                    wrs = sp.tile([P, ET], BF16, tag=f"wrs{q}")
                    nc.vector.tensor_copy(wrs[:], wrp[:])
                    rq = sp.tile([P, ET], BF16, tag=f"rq{q}")
                    nc.vector.tensor_tensor(out=rq[:], in0=wrs[:], in1=aiD[:],
                                            op=ALU.mult)
                    rqs.append(rq)
                tms = []
                for m in range(2):
                    cps = pa.tile([112, ET], F32, space="PSUM", tag="pa")
                    for q in range(4):
                        for a, b in NSL:
                            nc.tensor.matmul(cps[:, a:b],
                                             W4pt[:, q * 224 + m * 112:
                                                  q * 224 + (m + 1) * 112],
                                             rqs[q][:, a:b],
                                             start=(q == 0), stop=(q == 3))
                    cs = sp.tile([112, ET], BF16, tag=f"cs{m}")
                    nc.vector.tensor_copy(cs[:], cps[:])
                    tm = sp.tile([112, ET], BF16, tag=f"tm{m}")
                    nc.vector.tensor_tensor(out=tm[:], in0=cs[:], in1=aiS[0:112, :],
                                            op=ALU.mult)
                    tms.append(tm)
                fps0 = pa.tile([16, ET], F32, space="PSUM", tag="pa")
                for a, b in NSL:
                    nc.tensor.matmul(fps0[:, a:b], L20[:], tms[0][:, a:b],
                                     start=True, stop=False)
                    nc.tensor.matmul(fps0[:, a:b], L2x[:], tms[1][:, a:b],
                                     start=False, stop=True)
                fps1 = pa.tile([44, ET], F32, space="PSUM", tag="pa")
                for a, b in NSL:
                    nc.tensor.matmul(fps1[:, a:b], L21[:], tms[1][:, a:b],
                                     start=True, stop=True)
                F = sp.tile([96, ET], BF16, tag="F")
                nc.gpsimd.memset(F[:], 0.0)
                nc.vector.tensor_copy(F[0:16, :], fps0[:])
                ss = sp.tile([44, ET], BF16, tag="ss")
                nc.vector.tensor_copy(ss[:], fps1[:])
                nc.vector.tensor_tensor(out=F[32:64, :], in0=ss[0:32, :],
                                        in1=shs[0:32, :], op=ALU.mult)
                nc.vector.tensor_tensor(out=F[64:76, :], in0=ss[32:44, :],
                                        in1=shs[32:44, :], op=ALU.mult)
                # scatter
                for cc in range(TILE_CH):
                    cg = t * TILE_CH + cc
                    win = cg // W_CH
                    ftp = pp.tile([P, 96], BF16, space="PSUM", tag="pcb")
                    nc.tensor.transpose(ftp[:], F[:, cc * P:(cc + 1) * P],
                                        identb[0:96, 0:96])
                    fc = sp.tile([P, 60], BF16, tag="fc")
                    nc.vector.tensor_copy(fc[:, 0:16], ftp[:, 0:16])
                    nc.vector.tensor_copy(fc[:, 16:60], ftp[:, 32:76])
                    wc = pp.tile([P, 60], F32, space="PSUM", tag="pc")
                    nc.tensor.matmul(wc[:], oht[:, cc * P:(cc + 1) * P], fc[:],
                                     start=True, stop=True)
                    if cg % W_CH == 0:
                        win_sb = sp.tile([P, 60], F32, tag="winsb")
                        nc.vector.tensor_copy(win_sb[:], wc[:])
                    else:
                        nc.vector.tensor_tensor(out=win_sb[:], in0=win_sb[:],
                                                in1=wc[:], op=ALU.add)
                    if cg % W_CH == W_CH - 1:
                        nc.sync.dma_start(out_d[win * P:(win + 1) * P, :],
                                          win_sb[:])
    nc.compile()
    return nc


_CACHE = {}


def kernel(**inputs):
    per_core, onehotA, recip_pl, W_CH, C_TOT, E = _host_prep(inputs)
    et = np.asarray(inputs["embed_table"], np.float32)
    fW4 = np.asarray(inputs["fc_W4"], np.float32)
    consts = _build_consts(fW4)
    W4p, Sel, L2_0, L2_x, L2_1, Selsh = consts
    key = (W_CH, C_TOT)
    if key not in _CACHE:
        _CACHE[key] = _build_bass(W_CH, C_TOT, E, consts)
    nc = _CACHE[key]
    shared = dict(
        onehotA=onehotA, recip_pl=recip_pl,
        TA=(et @ np.asarray(inputs["fit_W1"], np.float32)).astype(np.float32),
        fit_W2=np.asarray(inputs["fit_W2"], np.float32),
        fit_W3=np.asarray(inputs["fit_W3"], np.float32),
        fc_W1p=(np.asarray(inputs["fc_W1"], np.float32) / 1.12),
        fc_W2p=(np.asarray(inputs["fc_W2"], np.float32) / 8.0),
        fc_W3p=(np.asarray(inputs["fc_W3"], np.float32) / 8.0),
        W4p=np.ascontiguousarray(np.transpose(W4p.reshape(4, 128, 224), (1, 0, 2)).reshape(128, 896)),
        cvec=np.tile(np.array([*(-VCENters / STEP), 1.0 / STEP, -1.0,
                               15.0 ** 0.5, 0.5 * 5.0 ** 0.5, 5.0 ** 0.5,
                               0.5 * 15.0 ** 0.5], np.float32), (P, 1)),
        Sel=np.ascontiguousarray(np.transpose(Sel, (1, 0, 2)).reshape(64, 512)),
        L2_0=L2_0, L2_x=L2_x, L2_1=L2_1, Selsh=Selsh,
    )
    import ml_dtypes
    for k in ("W4p", "Sel", "L2_0", "L2_x", "L2_1", "TA", "fit_W2", "fit_W3",
              "fc_W1p", "fc_W2p", "fc_W3p", "Selsh", "onehotA"):
        shared[k] = shared[k].astype(ml_dtypes.bfloat16)
    in_maps = []
    for ci in range(N_CORES):
        m = dict(shared)
        m.update(geom_pl=per_core[ci]["geom_pl"], dst_pl=per_core[ci]["dst_pl"],
                 oh_pl=per_core[ci]["oh_pl"].astype(ml_dtypes.bfloat16),
                 idx_src=per_core[ci]["idx_src"], idx_dst=per_core[ci]["idx_dst"])
        in_maps.append(m)
    res = bass_utils.run_bass_kernel_spmd(nc, in_maps, core_ids=list(range(N_CORES)))
    out = np.concatenate([res.results[ci]["out"][:NPC] for ci in range(N_CORES)], 0)
    return out.astype(np.float32)


# revision 36
# speedup vs baseline: 1.0598x; 1.0598x over previous
"""E3Conv Trainium2 kernel: 8-core SPMD, dst-partitioned edges.

Strategy: sort edges by dst; core i owns nodes [1250i,1250(i+1)) and all edges
into them (no all-reduce needed). Per core: node-MLP replicated, bf16 gather
tables for Ai/Ai*recip, radial MLP + tensor-product restructured as one
K=512 matmul per edge tile, windowed one-hot matmul scatter-mean.
"""
import sys, os
sys.path.insert(0, "/opt/trn_rl_repo")
import numpy as np

import concourse.bass as bass
import concourse.tile as tile
from concourse import bacc, mybir
from concourse import bass_utils
from concourse.masks import make_identity

P = 128
N_NODES, N_EDGES, N_GRAPHS = 10000, 131072, 64
N_CORES, NPC, N_WIN = 8, 1250, 10
MAX_RADIUS, N_BASIS = 4.0, 10
STEP = MAX_RADIUS / (N_BASIS + 1)
VCENters = np.linspace(0.0, MAX_RADIUS, N_BASIS + 2)[1:-1].astype(np.float32)
F32, BF16, F32R, I16 = (mybir.dt.float32, mybir.dt.bfloat16,
                        mybir.dt.float32r, mybir.dt.int16)
AF = mybir.ActivationFunctionType
ALU = mybir.AluOpType
NCH = 79  # node chunks of 128 (79*128 = 10112 >= 10000)


def _build_consts(fW4):
    s3 = 3.0 ** 0.5
    W4p = np.zeros((512, 224), np.float32)
    offs = {0: 0, 1: 1024, 2: 1536}
    wbase = {0: 0, 1: 16, 2: 24}
    scale_l = {0: 1.0 / 64, 1: s3 / 64, 2: 1.0 / 64}
    for l, mo in enumerate((16, 8, 4)):
        for u in range(8):
            for v in range(8):
                for wl in range(mo):
                    col = offs[l] + (u * 8 + v) * mo + wl
                    w = wbase[l] + wl
                    W4p[np.arange(64) * 8 + v, w * 8 + u] = fW4[:, col] * scale_l[l]
    Sel = np.zeros((4, 64, 128), np.float32)
    for q in range(4):
        for r in range(128):
            Sel[q, 16 * q + r // 8, r] = 1.0
    L2_0 = np.zeros((112, 16), np.float32)
    for r in range(112):
        L2_0[r, r // 8] = 1.0
    L2_x = np.zeros((112, 16), np.float32)
    for r in range(16):
        L2_x[r, 14 + r // 8] = 1.0
    L2_1 = np.zeros((112, 44), np.float32)
    for r in range(112):
        w = 14 + r // 8
        if w < 16:
            pass
        elif w < 24:
            for m in range(3):
                L2_1[r, (w - 16) * 3 + m] = 1.0
        else:
            for m in range(5):
                L2_1[r, 24 + (w - 24) * 5 + m] = 1.0
    Selsh = np.zeros((8, 44), np.float32)
    for w in range(8):
        for m in range(3):
            Selsh[m, w * 3 + m] = 1.0
    for w in range(4):
        for m in range(5):
            Selsh[3 + m, 24 + w * 5 + m] = 1.0
    return W4p, Sel, L2_0, L2_x, L2_1, Selsh


def _host_prep(inputs):
    pos = np.asarray(inputs["pos"], np.float32)
    A = np.asarray(inputs["A"]).astype(np.int64)
    batch = np.asarray(inputs["batch"]).astype(np.int64)
    esrc = np.asarray(inputs["edge_src"]).astype(np.int64)
    edst = np.asarray(inputs["edge_dst"]).astype(np.int64)
    shifts = np.asarray(inputs["edge_shifts"], np.float32)
    cell = np.asarray(inputs["cell"], np.float32)
    counts = np.bincount(edst, minlength=N_NODES).astype(np.float32)
    recipc = 1.0 / np.maximum(counts, 1.0)
    cpn = cell[batch].reshape(N_NODES, 9)
    order = np.argsort(edst, kind="stable")
    wins_all, W_CH = [], 0
    for ci in range(N_CORES):
        lo = ci * NPC
        m = order[(edst[order] >= lo) & (edst[order] < lo + NPC)]
        wins = []
        for w in range(N_WIN):
            wlo = lo + w * P
            whi = min(lo + (w + 1) * P, lo + NPC)
            wm = m[(edst[m] >= wlo) & (edst[m] < whi)]
            wins.append(wm)
            W_CH = max(W_CH, (len(wm) + P - 1) // P)
    # uniform even chunk count per window across all cores
        wins_all.append(wins)
    if W_CH % 2:
        W_CH += 1
    C_TOT = N_WIN * W_CH
    E = C_TOT * P
    onehotA = np.zeros((10, N_NODES), np.float32)
    onehotA[A, np.arange(N_NODES)] = 1.0
    recip_pl = np.ones((P, NCH), np.float32)
    rc = np.concatenate([recipc, np.ones(NCH * P - N_NODES, np.float32)])
    recip_pl[:, :] = rc.reshape(NCH, P).T
    per_core = []
    for ci in range(N_CORES):
        idx = np.zeros(E, np.int64)
        valid = np.zeros(E, bool)
        dstloc = np.full(E, -1.0, np.float32)
        for w in range(N_WIN):
            wm = wins_all[ci][w]
            s = w * W_CH * P
            idx[s:s + len(wm)] = wm
            valid[s:s + len(wm)] = True
            dstloc[s:s + len(wm)] = (edst[wm] - ci * NPC - w * P).astype(np.float32)
        src = np.where(valid, esrc[idx], 0)
        dst = np.where(valid, edst[idx], 0)
        sh = np.where(valid[:, None], shifts[idx], np.float32(1.0))
        geom = np.concatenate([pos[src], pos[dst], sh, cpn[src]], 1)  # [E,18]
        geom_pl = np.ascontiguousarray(
            np.transpose(geom.reshape(C_TOT, P, 18), (1, 2, 0)).reshape(P, 18 * C_TOT))
        dst_pl = np.ascontiguousarray(dstloc.reshape(C_TOT, P).T)

        def wrap(ix):
            wr = ix.astype(np.int16).reshape(-1, 16).T  # [16, E/16]
            return np.ascontiguousarray(np.tile(wr, (8, 1)))
        ohm = (dstloc.reshape(C_TOT, P, 1) ==
               np.arange(P, dtype=np.float32)[None, None, :])
        oh_pl = np.ascontiguousarray(
            np.transpose(ohm, (1, 0, 2)).reshape(P, C_TOT * P))
        per_core.append(dict(geom_pl=geom_pl, dst_pl=dst_pl, oh_pl=oh_pl,
                             idx_src=wrap(src), idx_dst=wrap(dst)))
    return per_core, onehotA, recip_pl, W_CH, C_TOT, E


def _build_bass(W_CH, C_TOT, E, consts):
    W4p, Sel, L2_0, L2_x, L2_1, Selsh = [c.astype(np.float32) for c in consts[:6]]
    TILE_CH = W_CH // 2
    NT = C_TOT // TILE_CH
    ET = TILE_CH * P
    NIW = E // 16
    nc = bacc.Bacc("TRN2", target_bir_lowering=False, debug=False,
                   num_devices=N_CORES)

    def din(name, shape, dt=F32):
        return nc.dram_tensor(name, shape, dt, kind="ExternalInput").ap()

    geom_d = din("geom_pl", [P, 18 * C_TOT])
    dst_d = din("dst_pl", [P, C_TOT])
    ohm_d = din("oh_pl", [P, C_TOT * P], BF16)
    isrc_d = din("idx_src", [P, NIW], I16)
    idst_d = din("idx_dst", [P, NIW], I16)
    ohA_d = din("onehotA", [10, N_NODES], BF16)
    rcp_d = din("recip_pl", [P, NCH])
    TA_d = din("TA", [10, 64], BF16)
    W2_d = din("fit_W2", [64, 32], BF16)
    W3_d = din("fit_W3", [32, 8], BF16)
    fW1_d = din("fc_W1p", [10, 64], BF16)
    fW2_d = din("fc_W2p", [64, 64], BF16)
    fW3_d = din("fc_W3p", [64, 64], BF16)
    W4p_d = din("W4p", [128, 4 * 224], BF16)
    Sel_d = din("Sel", [64, 4 * 128], BF16)
    L20_d = din("L2_0", [112, 16], BF16)
    L2x_d = din("L2_x", [112, 16], BF16)
    L21_d = din("L2_1", [112, 44], BF16)
    Ssh_d = din("Selsh", [8, 44], BF16)
    cv_d = din("cvec", [P, 16])
    out_d = nc.dram_tensor("out", [N_WIN * P, 60], F32, kind="ExternalOutput").ap()

    C = C_TOT
    with tile.TileContext(nc) as tc:
        with tc.tile_pool(name="const", bufs=1) as cp, \
             tc.tile_pool(name="sb", bufs=2) as sp, \
             tc.tile_pool(name="big", bufs=1) as bp, \
             tc.tile_pool(name="ps", bufs=1, space="PSUM") as pp, \
             tc.tile_pool(name="psa", bufs=2, space="PSUM") as pa, \
             tc.tile_pool(name="pswin", bufs=2, space="PSUM") as pw, \
             tc.tile_pool(name="dram", bufs=1, space="DRAM") as dp:
            ident = cp.tile([P, P], F32)
            make_identity(nc, ident[:])
            identb = cp.tile([P, P], BF16)
            nc.vector.tensor_copy(identb[:], ident[:])
            io_i = cp.tile([P, P], mybir.dt.int32)
            nc.gpsimd.iota(io_i[:], pattern=[[1, P]], base=0, channel_multiplier=0)
            io_f = cp.tile([P, P], F32)
            nc.vector.tensor_copy(io_f[:], io_i[:])

            def load_const(dram, shape, dt=F32):
                t = cp.tile(shape, dt, tag=dram.tensor.name)
                nc.sync.dma_start(t[:], dram[:])
                return t
            TA = load_const(TA_d, [10, 64], BF16)
            W2 = load_const(W2_d, [64, 32], BF16)
            W3 = load_const(W3_d, [32, 8], BF16)
            fW1 = load_const(fW1_d, [10, 64], BF16)
            fW2 = load_const(fW2_d, [64, 64], BF16)
            fW3 = load_const(fW3_d, [64, 64], BF16)
            W4pt = load_const(W4p_d, [128, 4 * 224], BF16)
            Selt = load_const(Sel_d, [64, 4 * 128], BF16)
            L20 = load_const(L20_d, [112, 16], BF16)
            L2x = load_const(L2x_d, [112, 16], BF16)
            L21 = load_const(L21_d, [112, 44], BF16)
            Ssh = load_const(Ssh_d, [8, 44], BF16)
            cv = load_const(cv_d, [P, 16])
            rcp = load_const(rcp_d, [P, NCH])
            ohA = bp.tile([10, N_NODES], BF16)
            nc.sync.dma_start(ohA[:], ohA_d[:])
            dstl = bp.tile([P, C], F32)
            nc.sync.dma_start(dstl[:], dst_d[:])
            isrc = bp.tile([P, NIW], I16)
            nc.sync.dma_start(isrc[:], isrc_d[:])
            idst = bp.tile([P, NIW], I16)
            nc.sync.dma_start(idst[:], idst_d[:])


            # ---------------- node MLP + gather tables ----------------
            Tsrc = dp.tile([NCH * P, P], BF16)
            Tdst = dp.tile([NCH * P, P], BF16)
            Ai_sb = bp.tile([8, NCH * P], F32)
            nc.gpsimd.memset(Ai_sb[:], 0.0)
            for j in range(20):
                s = j * 512
                n = min(512, N_NODES - s)
                h1p = pp.tile([64, 512], F32, space="PSUM", tag="pb")
                nc.tensor.matmul(h1p[:, :n], TA[:], ohA[:, s:s + n],
                                 start=True, stop=True)
                h1 = sp.tile([64, 512], BF16, tag="h1n")
                nc.scalar.activation(h1[:, :n], h1p[:, :n], AF.Silu)
                h2p = pp.tile([32, 512], F32, space="PSUM", tag="pb")
                nc.tensor.matmul(h2p[:, :n], W2[:], h1[:, :n],
                                 start=True, stop=True)
                h2 = sp.tile([32, 512], BF16, tag="h2n")
                nc.scalar.activation(h2[:, :n], h2p[:, :n], AF.Silu)
                aip = pa.tile([8, 512], F32, space="PSUM", tag="pa")
                nc.tensor.matmul(aip[:, :n], W3[:], h2[:, :n],
                                 start=True, stop=True)
                nc.vector.tensor_copy(Ai_sb[:, s:s + n], aip[:, :n])
            for c in range(NCH):
                s = c * P
                tp = pp.tile([P, 8], F32, space="PSUM", tag="pc")
                nc.tensor.transpose(tp[:], Ai_sb[:, s:s + P], ident[0:8, 0:8])
                f16a = sp.tile([P, 8], BF16, tag="f16a")
                nc.vector.tensor_copy(f16a[:], tp[:])
                f16b = sp.tile([P, 8], BF16, tag="f16b")
                nc.vector.tensor_tensor(
                    out=f16b[:], in0=tp[:],
                    in1=rcp[:, c:c + 1].to_broadcast([P, 8]), op=ALU.mult)
                repa = sp.tile([P, P], BF16, tag="repa")
                nc.vector.tensor_copy(
                    repa[:].rearrange("p (r v) -> p r v", v=8),
                    f16a[:].unsqueeze(1).to_broadcast([P, 16, 8]))
                repb = sp.tile([P, P], BF16, tag="repb")
                nc.vector.tensor_copy(
                    repb[:].rearrange("p (r v) -> p r v", v=8),
                    f16b[:].unsqueeze(1).to_broadcast([P, 16, 8]))
                nc.sync.dma_start(Tsrc[s:s + P, :], repa[:])
                nc.sync.dma_start(Tdst[s:s + P, :], repb[:])

            # ---------------- geometry (plane layout, whole E) ----------------
            gm = bp.tile([P, 18 * C], F32)
            nc.sync.dma_start(gm[:], geom_d[:])
            g3 = gm[:].rearrange("p (f c) -> p f c", f=18)
            tmp9 = bp.tile([P, 9 * C], F32)
            nc.vector.tensor_tensor(
                out=tmp9[:].rearrange("p (i j c) -> p i j c", i=3, j=3),
                in0=gm[:, 9 * C:18 * C].rearrange("p (i j c) -> p i j c", i=3, j=3),
                in1=gm[:, 6 * C:9 * C].rearrange("p (i c) -> p i c", i=3)
                    .unsqueeze(2).to_broadcast([P, 3, 3, C]),
                op=ALU.mult)
            sv = bp.tile([P, 3 * C], F32)
            nc.vector.tensor_tensor(out=sv[:], in0=tmp9[:, 0:3 * C],
                                    in1=tmp9[:, 3 * C:6 * C], op=ALU.add)
            nc.vector.tensor_tensor(out=sv[:], in0=sv[:],
                                    in1=tmp9[:, 6 * C:9 * C], op=ALU.add)
            ev = bp.tile([P, 3 * C], F32)
            nc.vector.tensor_tensor(out=ev[:], in0=g3[:, 3:6].rearrange("p f c -> p (f c)"),
                                    in1=g3[:, 0:3].rearrange("p f c -> p (f c)"),
                                    op=ALU.subtract)
            nc.vector.tensor_tensor(out=ev[:], in0=ev[:], in1=sv[:], op=ALU.add)
            sq = bp.tile([P, 3 * C], F32)
            nc.vector.tensor_tensor(out=sq[:], in0=ev[:], in1=ev[:], op=ALU.mult)
            ln2 = bp.tile([P, C], F32)
            nc.vector.tensor_tensor(out=ln2[:], in0=sq[:, 0:C], in1=sq[:, C:2 * C],
                                    op=ALU.add)
            nc.vector.tensor_tensor(out=ln2[:], in0=ln2[:], in1=sq[:, 2 * C:3 * C],
                                    op=ALU.add)
            ln = bp.tile([P, C], F32)
            nc.scalar.activation(ln[:], ln2[:], AF.Sqrt)
            rl = bp.tile([P, C], F32)
            nc.vector.reciprocal(rl[:], ln[:])
            u = bp.tile([P, 3 * C], F32)
            nc.vector.tensor_tensor(
                out=u[:].rearrange("p (f c) -> p f c", f=3),
                in0=ev[:].rearrange("p (f c) -> p f c", f=3),
                in1=rl[:].unsqueeze(1).to_broadcast([P, 3, C]), op=ALU.mult)
            usq = bp.tile([P, 3 * C], F32)
            nc.vector.tensor_tensor(out=usq[:], in0=u[:], in1=u[:], op=ALU.mult)
            # feature planes tile: f-major [basis10 | sh1 3 | sh2 5]
            gf = bp.tile([P, 18 * C], F32)
            dt2 = bp.tile([P, 10 * C], F32)
            s5, s15 = 5.0 ** 0.5, 15.0 ** 0.5
            for b in range(N_BASIS):
                nc.scalar.activation(dt2[:, b * C:(b + 1) * C], ln[:], AF.Square,
                                     bias=cv[:, b:b + 1],
                                     scale=cv[:, 10:11])
            nc.scalar.activation(gf[:, 0:10 * C], dt2[:], AF.Exp,
                                 scale=cv[:, 11:12])
            nc.vector.tensor_copy(gf[:, 10 * C:13 * C], u[:])
            t1 = bp.tile([P, C], F32)
            nc.scalar.mul(t1[:], u[:, 2 * C:3 * C], cv[:, 12:13])       # sqrt15*uz
            nc.vector.tensor_tensor(out=gf[:, 13 * C:14 * C], in0=u[:, 0:C],
                                    in1=t1[:], op=ALU.mult)     # m0
            nc.vector.tensor_tensor(out=gf[:, 16 * C:17 * C], in0=u[:, C:2 * C],
                                    in1=t1[:], op=ALU.mult)     # m3
            nc.scalar.mul(t1[:], u[:, 0:C], cv[:, 12:13])               # sqrt15*ux
            nc.vector.tensor_tensor(out=gf[:, 14 * C:15 * C], in0=u[:, C:2 * C],
                                    in1=t1[:], op=ALU.mult)     # m1
            t2 = bp.tile([P, C], F32)
            nc.vector.tensor_tensor(out=t2[:], in0=usq[:, 0:C],
                                    in1=usq[:, 2 * C:3 * C], op=ALU.add)
            nc.scalar.mul(t2[:], t2[:], cv[:, 13:14])
            t3 = bp.tile([P, C], F32)
            nc.scalar.mul(t3[:], usq[:, C:2 * C], cv[:, 14:15])
            nc.vector.tensor_tensor(out=gf[:, 15 * C:16 * C], in0=t3[:], in1=t2[:],
                                    op=ALU.subtract)            # m2
            nc.vector.tensor_tensor(out=t2[:], in0=usq[:, 2 * C:3 * C],
                                    in1=usq[:, 0:C], op=ALU.subtract)
            nc.scalar.mul(gf[:, 17 * C:18 * C], t2[:], cv[:, 15:16])  # m4
            gfv = gf[:].rearrange("p (f c) -> p f c", f=18)

            NSL = [(0, 512), (512, ET)] if ET > 512 else [(0, ET)]
            # ---------------- edge tiles ----------------
            win_sb = None
            for t in range(NT):
                cols = slice(t * ET, (t + 1) * ET)
                wcols = slice(t * (NIW // NT), (t + 1) * (NIW // NT))
                aiS = sp.tile([P, ET], BF16, tag="aiS")
                nc.gpsimd.dma_gather(
                    aiS[:].unsqueeze(1), Tsrc[:, :], isrc[:, wcols], ET, ET, P,
                    transpose=True)
                aiD = sp.tile([P, ET], BF16, tag="aiD")
                nc.gpsimd.dma_gather(
                    aiD[:].unsqueeze(1), Tdst[:, :], idst[:, wcols], ET, ET, P,
                    transpose=True)
                oht = sp.tile([P, ET], BF16, tag="oht")
                nc.sync.dma_start(oht[:], ohm_d[:, t * ET:(t + 1) * ET])
                h1p = pa.tile([64, ET], F32, space="PSUM", tag="pa")
                bts = []
                for cc in range(TILE_CH):
                    cg = t * TILE_CH + cc
                    btp = pp.tile([10, P], F32, space="PSUM", tag="pc")
                    nc.tensor.transpose(btp[:], gfv[:, 0:10, cg], ident[:])
                    bt = sp.tile([10, P], BF16, tag=f"bt{cc}")
                    nc.vector.tensor_copy(bt[:], btp[:])
                    btq = pp.tile([8, P], F32, space="PSUM", tag="pc")
                    nc.tensor.transpose(btq[:], gfv[:, 10:18, cg], ident[:])
                    bsh = sp.tile([8, P], BF16, tag=f"bsh{cc}")
                    nc.vector.tensor_copy(bsh[:], btq[:])
                    bts.append(bsh)
                    csl = slice(cc * P, (cc + 1) * P)
                    nc.tensor.matmul(h1p[:, csl], fW1[:], bt[:],
                                     start=True, stop=True)
                h1 = sp.tile([64, ET], BF16, tag="eh1")
                nc.scalar.activation(h1[:], h1p[:], AF.Silu)
                shp = pa.tile([44, ET], F32, space="PSUM", tag="pa")
                for cc in range(TILE_CH):
                    csl = slice(cc * P, (cc + 1) * P)
                    nc.tensor.matmul(shp[:, csl], Ssh[:], bts[cc][:],
                                     start=True, stop=True)
                shs = sp.tile([44, ET], BF16, tag="shs")
                nc.vector.tensor_copy(shs[:], shp[:])
                h2p = pa.tile([64, ET], F32, space="PSUM", tag="pa")
                for a, b in NSL:
                    nc.tensor.matmul(h2p[:, a:b], fW2[:], h1[:, a:b],
                                     start=True, stop=True)
                h2 = sp.tile([64, ET], BF16, tag="eh2")
                nc.scalar.activation(h2[:], h2p[:], AF.Silu)
                h3p = pa.tile([64, ET], F32, space="PSUM", tag="pa")
                for a, b in NSL:
                    nc.tensor.matmul(h3p[:, a:b], fW3[:], h2[:, a:b],
                                     start=True, stop=True)
                w3b = sp.tile([64, ET], BF16, tag="ew3b")
                nc.scalar.activation(w3b[:], h3p[:], AF.Silu)
                rqs = []
                for q in range(4):
                    wrp = pa.tile([P, ET], F32, space="PSUM", tag="pa")
                    for a, b in NSL:
                        nc.tensor.matmul(wrp[:, a:b],
                                         Selt[:, 128 * q:128 * (q + 1)],
                                         w3b[:, a:b], start=True, stop=True)
                    wrs = sp.tile([P, ET], BF16, tag=f"wrs{q}")
                    nc.vector.tensor_copy(wrs[:], wrp[:])
                    rq = sp.tile([P, ET], BF16, tag=f"rq{q}")
                    nc.vector.tensor_tensor(out=rq[:], in0=wrs[:], in1=aiD[:],
                                            op=ALU.mult)
                    rqs.append(rq)
                tms = []
                for m in range(2):
                    cps = pa.tile([112, ET], F32, space="PSUM", tag="pa")
                    for q in range(4):
                        for a, b in NSL:
                            nc.tensor.matmul(cps[:, a:b],
                                             W4pt[:, q * 224 + m * 112:
                                                  q * 224 + (m + 1) * 112],
                                             rqs[q][:, a:b],
                                             start=(q == 0), stop=(q == 3))
                    cs = sp.tile([112, ET], BF16, tag=f"cs{m}")
                    nc.vector.tensor_copy(cs[:], cps[:])
                    tm = sp.tile([112, ET], BF16, tag=f"tm{m}")
                    nc.vector.tensor_tensor(out=tm[:], in0=cs[:], in1=aiS[0:112, :],
                                            op=ALU.mult)
                    tms.append(tm)
                fps0 = pa.tile([16, ET], F32, space="PSUM", tag="pa")
                for a, b in NSL:
                    nc.tensor.matmul(fps0[:, a:b], L20[:], tms[0][:, a:b],
                                     start=True, stop=False)
                    nc.tensor.matmul(fps0[:, a:b], L2x[:], tms[1][:, a:b],
                                     start=False, stop=True)
                fps1 = pa.tile([44, ET], F32, space="PSUM", tag="pa")
                for a, b in NSL:
                    nc.tensor.matmul(fps1[:, a:b], L21[:], tms[1][:, a:b],
                                     start=True, stop=True)
                F = sp.tile([96, ET], BF16, tag="F")
                nc.gpsimd.memset(F[:], 0.0)
                nc.vector.tensor_copy(F[0:16, :], fps0[:])
                ss = sp.tile([44, ET], BF16, tag="ss")
                nc.vector.tensor_copy(ss[:], fps1[:])
                nc.vector.tensor_tensor(out=F[32:64, :], in0=ss[0:32, :],
                                        in1=shs[0:32, :], op=ALU.mult)
                nc.vector.tensor_tensor(out=F[64:76, :], in0=ss[32:44, :],
                                        in1=shs[32:44, :], op=ALU.mult)
                # scatter
                for cc in range(TILE_CH):
                    cg = t * TILE_CH + cc
                    win = cg // W_CH
                    ftp = pp.tile([P, 96], BF16, space="PSUM", tag="pcb")
                    nc.tensor.transpose(ftp[:], F[:, cc * P:(cc + 1) * P],
                                        identb[0:96, 0:96])
                    fc = sp.tile([P, 60], BF16, tag="fc")
                    nc.vector.tensor_copy(fc[:, 0:16], ftp[:, 0:16])
                    nc.vector.tensor_copy(fc[:, 16:60], ftp[:, 32:76])
                    wc = pp.tile([P, 60], F32, space="PSUM", tag="pc")
                    nc.tensor.matmul(wc[:], oht[:, cc * P:(cc + 1) * P], fc[:],
                                     start=True, stop=True)
                    if cg % W_CH == 0:
                        win_sb = sp.tile([P, 60], F32, tag="winsb")
                        nc.vector.tensor_copy(win_sb[:], wc[:])
                    else:
                        nc.vector.tensor_tensor(out=win_sb[:], in0=win_sb[:],
                                                in1=wc[:], op=ALU.add)
                    if cg % W_CH == W_CH - 1:
                        nc.sync.dma_start(out_d[win * P:(win + 1) * P, :],
                                          win_sb[:])
    nc.compile()
    return nc


_CACHE = {}


def kernel(**inputs):
    per_core, onehotA, recip_pl, W_CH, C_TOT, E = _host_prep(inputs)
    et = np.asarray(inputs["embed_table"], np.float32)
    fW4 = np.asarray(inputs["fc_W4"], np.float32)
    consts = _build_consts(fW4)
    W4p, Sel, L2_0, L2_x, L2_1, Selsh = consts
    key = (W_CH, C_TOT)
    if key not in _CACHE:
        _CACHE[key] = _build_bass(W_CH, C_TOT, E, consts)
    nc = _CACHE[key]
    shared = dict(
        onehotA=onehotA, recip_pl=recip_pl,
        TA=(et @ np.asarray(inputs["fit_W1"], np.float32)).astype(np.float32),
        fit_W2=np.asarray(inputs["fit_W2"], np.float32),
        fit_W3=np.asarray(inputs["fit_W3"], np.float32),
        fc_W1p=(np.asarray(inputs["fc_W1"], np.float32) / 1.12),
        fc_W2p=(np.asarray(inputs["fc_W2"], np.float32) / 8.0),
        fc_W3p=(np.asarray(inputs["fc_W3"], np.float32) / 8.0),
        W4p=np.ascontiguousarray(np.transpose(W4p.reshape(4, 128, 224), (1, 0, 2)).reshape(128, 896)),
        cvec=np.tile(np.array([*(-VCENters / STEP), 1.0 / STEP, -1.0,
                               15.0 ** 0.5, 0.5 * 5.0 ** 0.5, 5.0 ** 0.5,
                               0.5 * 15.0 ** 0.5], np.float32), (P, 1)),
        Sel=np.ascontiguousarray(np.transpose(Sel, (1, 0, 2)).reshape(64, 512)),
        L2_0=L2_0, L2_x=L2_x, L2_1=L2_1, Selsh=Selsh,
    )
    import ml_dtypes
    for k in ("W4p", "Sel", "L2_0", "L2_x", "L2_1", "TA", "fit_W2", "fit_W3",
              "fc_W1p", "fc_W2p", "fc_W3p", "Selsh", "onehotA"):
        shared[k] = shared[k].astype(ml_dtypes.bfloat16)
    in_maps = []
    for ci in range(N_CORES):
        m = dict(shared)
        m.update(geom_pl=per_core[ci]["geom_pl"], dst_pl=per_core[ci]["dst_pl"],
                 oh_pl=per_core[ci]["oh_pl"].astype(ml_dtypes.bfloat16),
                 idx_src=per_core[ci]["idx_src"], idx_dst=per_core[ci]["idx_dst"])
        in_maps.append(m)
    res = bass_utils.run_bass_kernel_spmd(nc, in_maps, core_ids=list(range(N_CORES)))
    out = np.concatenate([res.results[ci]["out"][:NPC] for ci in range(N_CORES)], 0)
    return out.astype(np.float32)


# revision 37
# speedup vs baseline: 1.1097x; 1.0471x over previous
"""E3Conv Trainium2 kernel: 8-core SPMD, dst-partitioned edges.

Strategy: sort edges by dst; core i owns nodes [1250i,1250(i+1)) and all edges
into them (no all-reduce needed). Per core: node-MLP replicated, bf16 gather
tables for Ai/Ai*recip, radial MLP + tensor-product restructured as one
K=512 matmul per edge tile, windowed one-hot matmul scatter-mean.
"""
import sys, os
sys.path.insert(0, "/opt/trn_rl_repo")
import numpy as np

import concourse.bass as bass
import concourse.tile as tile
from concourse import bacc, mybir
from concourse import bass_utils
from concourse.masks import make_identity

P = 128
N_NODES, N_EDGES, N_GRAPHS = 10000, 131072, 64
N_CORES, NPC, N_WIN = 8, 1250, 10
MAX_RADIUS, N_BASIS = 4.0, 10
STEP = MAX_RADIUS / (N_BASIS + 1)
VCENters = np.linspace(0.0, MAX_RADIUS, N_BASIS + 2)[1:-1].astype(np.float32)
F32, BF16, F32R, I16 = (mybir.dt.float32, mybir.dt.bfloat16,
                        mybir.dt.float32r, mybir.dt.int16)
AF = mybir.ActivationFunctionType
ALU = mybir.AluOpType
NCH = 79  # node chunks of 128 (79*128 = 10112 >= 10000)


def _build_consts(fW4):
    s3 = 3.0 ** 0.5
    W4p = np.zeros((512, 224), np.float32)
    offs = {0: 0, 1: 1024, 2: 1536}
    wbase = {0: 0, 1: 16, 2: 24}
    scale_l = {0: 1.0 / 64, 1: s3 / 64, 2: 1.0 / 64}
    for l, mo in enumerate((16, 8, 4)):
        for u in range(8):
            for v in range(8):
                for wl in range(mo):
                    col = offs[l] + (u * 8 + v) * mo + wl
                    w = wbase[l] + wl
                    W4p[np.arange(64) * 8 + v, w * 8 + u] = fW4[:, col] * scale_l[l]
    Sel = np.zeros((4, 64, 128), np.float32)
    for q in range(4):
        for r in range(128):
            Sel[q, 16 * q + r // 8, r] = 1.0
    L2_0 = np.zeros((112, 16), np.float32)
    for r in range(112):
        L2_0[r, r // 8] = 1.0
    L2_x = np.zeros((112, 16), np.float32)
    for r in range(16):
        L2_x[r, 14 + r // 8] = 1.0
    L2_1 = np.zeros((112, 44), np.float32)
    for r in range(112):
        w = 14 + r // 8
        if w < 16:
            pass
        elif w < 24:
            for m in range(3):
                L2_1[r, (w - 16) * 3 + m] = 1.0
        else:
            for m in range(5):
                L2_1[r, 24 + (w - 24) * 5 + m] = 1.0
    Selsh = np.zeros((8, 44), np.float32)
    for w in range(8):
        for m in range(3):
            Selsh[m, w * 3 + m] = 1.0
    for w in range(4):
        for m in range(5):
            Selsh[3 + m, 24 + w * 5 + m] = 1.0
    return W4p, Sel, L2_0, L2_x, L2_1, Selsh


def _host_prep(inputs):
    pos = np.asarray(inputs["pos"], np.float32)
    A = np.asarray(inputs["A"]).astype(np.int64)
    batch = np.asarray(inputs["batch"]).astype(np.int64)
    esrc = np.asarray(inputs["edge_src"]).astype(np.int64)
    edst = np.asarray(inputs["edge_dst"]).astype(np.int64)
    shifts = np.asarray(inputs["edge_shifts"], np.float32)
    cell = np.asarray(inputs["cell"], np.float32)
    counts = np.bincount(edst, minlength=N_NODES).astype(np.float32)
    recipc = 1.0 / np.maximum(counts, 1.0)
    cpn = cell[batch].reshape(N_NODES, 9)
    order = np.argsort(edst, kind="stable")
    wins_all, W_CH = [], 0
    for ci in range(N_CORES):
        lo = ci * NPC
        m = order[(edst[order] >= lo) & (edst[order] < lo + NPC)]
        wins = []
        for w in range(N_WIN):
            wlo = lo + w * P
            whi = min(lo + (w + 1) * P, lo + NPC)
            wm = m[(edst[m] >= wlo) & (edst[m] < whi)]
            wins.append(wm)
            W_CH = max(W_CH, (len(wm) + P - 1) // P)
    # uniform even chunk count per window across all cores
        wins_all.append(wins)
    if W_CH % 2:
        W_CH += 1
    C_TOT = N_WIN * W_CH
    E = C_TOT * P
    onehotA = np.zeros((10, N_NODES), np.float32)
    onehotA[A, np.arange(N_NODES)] = 1.0
    recip_pl = np.ones((P, NCH), np.float32)
    rc = np.concatenate([recipc, np.ones(NCH * P - N_NODES, np.float32)])
    recip_pl[:, :] = rc.reshape(NCH, P).T
    per_core = []
    for ci in range(N_CORES):
        idx = np.zeros(E, np.int64)
        valid = np.zeros(E, bool)
        dstloc = np.full(E, -1.0, np.float32)
        for w in range(N_WIN):
            wm = wins_all[ci][w]
            s = w * W_CH * P
            idx[s:s + len(wm)] = wm
            valid[s:s + len(wm)] = True
            dstloc[s:s + len(wm)] = (edst[wm] - ci * NPC - w * P).astype(np.float32)
        src = np.where(valid, esrc[idx], 0)
        dst = np.where(valid, edst[idx], 0)
        sh = np.where(valid[:, None], shifts[idx], np.float32(1.0))
        geom = np.concatenate([pos[src], pos[dst], sh, cpn[src]], 1)  # [E,18]
        geom_pl = np.ascontiguousarray(
            np.transpose(geom.reshape(C_TOT, P, 18), (1, 2, 0)).reshape(P, 18 * C_TOT))
        dst_pl = np.ascontiguousarray(dstloc.reshape(C_TOT, P).T)

        def wrap(ix):
            wr = ix.astype(np.int16).reshape(-1, 16).T  # [16, E/16]
            return np.ascontiguousarray(np.tile(wr, (8, 1)))
        ohm = (dstloc.reshape(C_TOT, P, 1) ==
               np.arange(P, dtype=np.float32)[None, None, :])
        oh_pl = np.ascontiguousarray(
            np.transpose(ohm, (1, 0, 2)).reshape(P, C_TOT * P))
        per_core.append(dict(geom_pl=geom_pl, dst_pl=dst_pl, oh_pl=oh_pl,
                             idx_src=wrap(src), idx_dst=wrap(dst)))
    return per_core, onehotA, recip_pl, W_CH, C_TOT, E


def _build_bass(W_CH, C_TOT, E, consts):
    W4p, Sel, L2_0, L2_x, L2_1, Selsh = [c.astype(np.float32) for c in consts[:6]]
    TILE_CH = W_CH // 2
    NT = C_TOT // TILE_CH
    ET = TILE_CH * P
    NIW = E // 16
    nc = bacc.Bacc("TRN2", target_bir_lowering=False, debug=False,
                   num_devices=N_CORES)

    def din(name, shape, dt=F32):
        return nc.dram_tensor(name, shape, dt, kind="ExternalInput").ap()

    geom_d = din("geom_pl", [P, 18 * C_TOT])
    dst_d = din("dst_pl", [P, C_TOT])
    ohm_d = din("oh_pl", [P, C_TOT * P], BF16)
    isrc_d = din("idx_src", [P, NIW], I16)
    idst_d = din("idx_dst", [P, NIW], I16)
    ohA_d = din("onehotA", [10, N_NODES], BF16)
    rcp_d = din("recip_pl", [P, NCH])
    TA_d = din("TA", [10, 64], BF16)
    W2_d = din("fit_W2", [64, 32], BF16)
    W3_d = din("fit_W3", [32, 8], BF16)
    fW1_d = din("fc_W1p", [10, 64], BF16)
    fW2_d = din("fc_W2p", [64, 64], BF16)
    fW3_d = din("fc_W3p", [64, 64], BF16)
    W4p_d = din("W4p", [128, 4 * 224], BF16)
    Sel_d = din("Sel", [64, 4 * 128], BF16)
    L20_d = din("L2_0", [112, 16], BF16)
    L2x_d = din("L2_x", [112, 16], BF16)
    L21_d = din("L2_1", [112, 44], BF16)
    Ssh_d = din("Selsh", [8, 44], BF16)
    cv_d = din("cvec", [P, 16])
    out_d = nc.dram_tensor("out", [N_WIN * P, 60], F32, kind="ExternalOutput").ap()

    C = C_TOT
    with tile.TileContext(nc) as tc:
        with tc.tile_pool(name="const", bufs=1) as cp, \
             tc.tile_pool(name="sb", bufs=2) as sp, \
             tc.tile_pool(name="big", bufs=1) as bp, \
             tc.tile_pool(name="ps", bufs=1, space="PSUM") as pp, \
             tc.tile_pool(name="psa", bufs=2, space="PSUM") as pa, \
             tc.tile_pool(name="pswin", bufs=2, space="PSUM") as pw, \
             tc.tile_pool(name="dram", bufs=1, space="DRAM") as dp:
            ident = cp.tile([P, P], F32)
            make_identity(nc, ident[:])
            identb = cp.tile([P, P], BF16)
            nc.vector.tensor_copy(identb[:], ident[:])
            io_i = cp.tile([P, P], mybir.dt.int32)
            nc.gpsimd.iota(io_i[:], pattern=[[1, P]], base=0, channel_multiplier=0)
            io_f = cp.tile([P, P], F32)
            nc.vector.tensor_copy(io_f[:], io_i[:])

            def load_const(dram, shape, dt=F32):
                t = cp.tile(shape, dt, tag=dram.tensor.name)
                nc.sync.dma_start(t[:], dram[:])
                return t
            TA = load_const(TA_d, [10, 64], BF16)
            W2 = load_const(W2_d, [64, 32], BF16)
            W3 = load_const(W3_d, [32, 8], BF16)
            fW1 = load_const(fW1_d, [10, 64], BF16)
            fW2 = load_const(fW2_d, [64, 64], BF16)
            fW3 = load_const(fW3_d, [64, 64], BF16)
            W4pt = load_const(W4p_d, [128, 4 * 224], BF16)
            Selt = load_const(Sel_d, [64, 4 * 128], BF16)
            L20 = load_const(L20_d, [112, 16], BF16)
            L2x = load_const(L2x_d, [112, 16], BF16)
            L21 = load_const(L21_d, [112, 44], BF16)
            Ssh = load_const(Ssh_d, [8, 44], BF16)
            cv = load_const(cv_d, [P, 16])
            rcp = load_const(rcp_d, [P, NCH])
            ohA = bp.tile([10, N_NODES], BF16)
            nc.sync.dma_start(ohA[:], ohA_d[:])
            dstl = bp.tile([P, C], F32)
            nc.sync.dma_start(dstl[:], dst_d[:])
            isrc = bp.tile([P, NIW], I16)
            nc.sync.dma_start(isrc[:], isrc_d[:])
            idst = bp.tile([P, NIW], I16)
            nc.sync.dma_start(idst[:], idst_d[:])


            # ---------------- node MLP + gather tables ----------------
            Tsrc = dp.tile([NCH * P, P], BF16)
            Tdst = dp.tile([NCH * P, P], BF16)
            Ai_sb = bp.tile([8, NCH * P], F32)
            nc.gpsimd.memset(Ai_sb[:], 0.0)
            for j in range(20):
                s = j * 512
                n = min(512, N_NODES - s)
                h1p = pp.tile([64, 512], F32, space="PSUM", tag="pb")
                nc.tensor.matmul(h1p[:, :n], TA[:], ohA[:, s:s + n],
                                 start=True, stop=True)
                h1 = sp.tile([64, 512], BF16, tag="h1n")
                nc.scalar.activation(h1[:, :n], h1p[:, :n], AF.Silu)
                h2p = pp.tile([32, 512], F32, space="PSUM", tag="pb")
                nc.tensor.matmul(h2p[:, :n], W2[:], h1[:, :n],
                                 start=True, stop=True)
                h2 = sp.tile([32, 512], BF16, tag="h2n")
                nc.scalar.activation(h2[:, :n], h2p[:, :n], AF.Silu)
                aip = pa.tile([8, 512], F32, space="PSUM", tag="pa")
                nc.tensor.matmul(aip[:, :n], W3[:], h2[:, :n],
                                 start=True, stop=True)
                nc.vector.tensor_copy(Ai_sb[:, s:s + n], aip[:, :n])
            for c in range(NCH):
                s = c * P
                tp = pp.tile([P, 8], F32, space="PSUM", tag="pc")
                nc.tensor.transpose(tp[:], Ai_sb[:, s:s + P], ident[0:8, 0:8])
                f16a = sp.tile([P, 8], BF16, tag="f16a")
                nc.vector.tensor_copy(f16a[:], tp[:])
                f16b = sp.tile([P, 8], BF16, tag="f16b")
                nc.vector.tensor_tensor(
                    out=f16b[:], in0=tp[:],
                    in1=rcp[:, c:c + 1].to_broadcast([P, 8]), op=ALU.mult)
                repa = sp.tile([P, P], BF16, tag="repa")
                nc.vector.tensor_copy(
                    repa[:].rearrange("p (r v) -> p r v", v=8),
                    f16a[:].unsqueeze(1).to_broadcast([P, 16, 8]))
                repb = sp.tile([P, P], BF16, tag="repb")
                nc.vector.tensor_copy(
                    repb[:].rearrange("p (r v) -> p r v", v=8),
                    f16b[:].unsqueeze(1).to_broadcast([P, 16, 8]))
                nc.sync.dma_start(Tsrc[s:s + P, :], repa[:])
                nc.sync.dma_start(Tdst[s:s + P, :], repb[:])

            # ---------------- geometry (plane layout, whole E) ----------------
            gm = bp.tile([P, 18 * C], F32)
            nc.sync.dma_start(gm[:], geom_d[:])
            g3 = gm[:].rearrange("p (f c) -> p f c", f=18)
            tmp9 = bp.tile([P, 9 * C], F32)
            nc.vector.tensor_tensor(
                out=tmp9[:].rearrange("p (i j c) -> p i j c", i=3, j=3),
                in0=gm[:, 9 * C:18 * C].rearrange("p (i j c) -> p i j c", i=3, j=3),
                in1=gm[:, 6 * C:9 * C].rearrange("p (i c) -> p i c", i=3)
                    .unsqueeze(2).to_broadcast([P, 3, 3, C]),
                op=ALU.mult)
            sv = bp.tile([P, 3 * C], F32)
            nc.vector.tensor_tensor(out=sv[:], in0=tmp9[:, 0:3 * C],
                                    in1=tmp9[:, 3 * C:6 * C], op=ALU.add)
            nc.vector.tensor_tensor(out=sv[:], in0=sv[:],
                                    in1=tmp9[:, 6 * C:9 * C], op=ALU.add)
            ev = bp.tile([P, 3 * C], F32)
            nc.vector.tensor_tensor(out=ev[:], in0=g3[:, 3:6].rearrange("p f c -> p (f c)"),
                                    in1=g3[:, 0:3].rearrange("p f c -> p (f c)"),
                                    op=ALU.subtract)
            nc.vector.tensor_tensor(out=ev[:], in0=ev[:], in1=sv[:], op=ALU.add)
            sq = bp.tile([P, 3 * C], F32)
            nc.vector.tensor_tensor(out=sq[:], in0=ev[:], in1=ev[:], op=ALU.mult)
            ln2 = bp.tile([P, C], F32)
            nc.vector.tensor_tensor(out=ln2[:], in0=sq[:, 0:C], in1=sq[:, C:2 * C],
                                    op=ALU.add)
            nc.vector.tensor_tensor(out=ln2[:], in0=ln2[:], in1=sq[:, 2 * C:3 * C],
                                    op=ALU.add)
            ln = bp.tile([P, C], F32)
            nc.scalar.activation(ln[:], ln2[:], AF.Sqrt)
            rl = bp.tile([P, C], F32)
            nc.vector.reciprocal(rl[:], ln[:])
            u = bp.tile([P, 3 * C], F32)
            nc.vector.tensor_tensor(
                out=u[:].rearrange("p (f c) -> p f c", f=3),
                in0=ev[:].rearrange("p (f c) -> p f c", f=3),
                in1=rl[:].unsqueeze(1).to_broadcast([P, 3, C]), op=ALU.mult)
            usq = bp.tile([P, 3 * C], F32)
            nc.vector.tensor_tensor(out=usq[:], in0=u[:], in1=u[:], op=ALU.mult)
            # feature planes tile: f-major [basis10 | sh1 3 | sh2 5]
            gf = bp.tile([P, 18 * C], F32)
            dt2 = bp.tile([P, 10 * C], F32)
            s5, s15 = 5.0 ** 0.5, 15.0 ** 0.5
            for b in range(N_BASIS):
                nc.scalar.activation(dt2[:, b * C:(b + 1) * C], ln[:], AF.Square,
                                     bias=cv[:, b:b + 1],
                                     scale=cv[:, 10:11])
            nc.scalar.activation(gf[:, 0:10 * C], dt2[:], AF.Exp,
                                 scale=cv[:, 11:12])
            nc.vector.tensor_copy(gf[:, 10 * C:13 * C], u[:])
            t1 = bp.tile([P, C], F32)
            nc.scalar.mul(t1[:], u[:, 2 * C:3 * C], cv[:, 12:13])       # sqrt15*uz
            nc.vector.tensor_tensor(out=gf[:, 13 * C:14 * C], in0=u[:, 0:C],
                                    in1=t1[:], op=ALU.mult)     # m0
            nc.vector.tensor_tensor(out=gf[:, 16 * C:17 * C], in0=u[:, C:2 * C],
                                    in1=t1[:], op=ALU.mult)     # m3
            nc.scalar.mul(t1[:], u[:, 0:C], cv[:, 12:13])               # sqrt15*ux
            nc.vector.tensor_tensor(out=gf[:, 14 * C:15 * C], in0=u[:, C:2 * C],
                                    in1=t1[:], op=ALU.mult)     # m1
            t2 = bp.tile([P, C], F32)
            nc.vector.tensor_tensor(out=t2[:], in0=usq[:, 0:C],
                                    in1=usq[:, 2 * C:3 * C], op=ALU.add)
            nc.scalar.mul(t2[:], t2[:], cv[:, 13:14])
            t3 = bp.tile([P, C], F32)
            nc.scalar.mul(t3[:], usq[:, C:2 * C], cv[:, 14:15])
            nc.vector.tensor_tensor(out=gf[:, 15 * C:16 * C], in0=t3[:], in1=t2[:],
                                    op=ALU.subtract)            # m2
            nc.vector.tensor_tensor(out=t2[:], in0=usq[:, 2 * C:3 * C],
                                    in1=usq[:, 0:C], op=ALU.subtract)
            nc.scalar.mul(gf[:, 17 * C:18 * C], t2[:], cv[:, 15:16])  # m4
            gfv = gf[:].rearrange("p (f c) -> p f c", f=18)

            NSL = [(0, 512), (512, ET)] if ET > 512 else [(0, ET)]
            # ---------------- edge tiles ----------------
            win_sb = None
            for t in range(NT):
                cols = slice(t * ET, (t + 1) * ET)
                wcols = slice(t * (NIW // NT), (t + 1) * (NIW // NT))
                aiS = sp.tile([P, ET], BF16, tag="aiS")
                nc.gpsimd.dma_gather(
                    aiS[:].unsqueeze(1), Tsrc[:, :], isrc[:, wcols], ET, ET, P,
                    transpose=True)
                aiD = sp.tile([P, ET], BF16, tag="aiD")
                nc.gpsimd.dma_gather(
                    aiD[:].unsqueeze(1), Tdst[:, :], idst[:, wcols], ET, ET, P,
                    transpose=True)
                oht = sp.tile([P, ET], BF16, tag="oht")
                nc.sync.dma_start(oht[:], ohm_d[:, t * ET:(t + 1) * ET])
                h1p = pa.tile([64, ET], F32, space="PSUM", tag="pa")
                bts = []
                for cc in range(TILE_CH):
                    cg = t * TILE_CH + cc
                    btp = pp.tile([10, P], F32, space="PSUM", tag="pc")
                    nc.tensor.transpose(btp[:], gfv[:, 0:10, cg], ident[:])
                    bt = sp.tile([10, P], BF16, tag=f"bt{cc}")
                    nc.vector.tensor_copy(bt[:], btp[:])
                    btq = pp.tile([8, P], F32, space="PSUM", tag="pc")
                    nc.tensor.transpose(btq[:], gfv[:, 10:18, cg], ident[:])
                    bsh = sp.tile([8, P], BF16, tag=f"bsh{cc}")
                    nc.vector.tensor_copy(bsh[:], btq[:])
                    bts.append(bsh)
                    csl = slice(cc * P, (cc + 1) * P)
                    nc.tensor.matmul(h1p[:, csl], fW1[:], bt[:],
                                     start=True, stop=True)
                h1 = sp.tile([64, ET], BF16, tag="eh1")
                nc.scalar.activation(h1[:], h1p[:], AF.Silu)
                shp = pa.tile([44, ET], F32, space="PSUM", tag="pa")
                for cc in range(TILE_CH):
                    csl = slice(cc * P, (cc + 1) * P)
                    nc.tensor.matmul(shp[:, csl], Ssh[:], bts[cc][:],
                                     start=True, stop=True)
                shs = sp.tile([44, ET], BF16, tag="shs")
                nc.vector.tensor_copy(shs[:], shp[:])
                h2p = pa.tile([64, ET], F32, space="PSUM", tag="pa")
                for a, b in NSL:
                    nc.tensor.matmul(h2p[:, a:b], fW2[:], h1[:, a:b],
                                     start=True, stop=True)
                h2 = sp.tile([64, ET], BF16, tag="eh2")
                nc.scalar.activation(h2[:], h2p[:], AF.Silu)
                h3p = pa.tile([64, ET], F32, space="PSUM", tag="pa")
                for a, b in NSL:
                    nc.tensor.matmul(h3p[:, a:b], fW3[:], h2[:, a:b],
                                     start=True, stop=True)
                w3b = sp.tile([64, ET], BF16, tag="ew3b")
                nc.scalar.activation(w3b[:], h3p[:], AF.Silu)
                rqs = []
                for q in range(4):
                    wrp = pa.tile([P, ET], F32, space="PSUM", tag="pa")
                    for a, b in NSL:
                        nc.tensor.matmul(wrp[:, a:b],
                                         Selt[:, 128 * q:128 * (q + 1)],
                                         w3b[:, a:b], start=True, stop=True)
                    wrs = sp.tile([P, ET], BF16, tag=f"wrs{q}")
                    nc.vector.tensor_copy(wrs[:], wrp[:])
                    rq = sp.tile([P, ET], BF16, tag=f"rq{q}")
                    nc.vector.tensor_tensor(out=rq[:], in0=wrs[:], in1=aiD[:],
                                            op=ALU.mult)
                    rqs.append(rq)
                tms = []
                for m in range(2):
                    cps = pa.tile([112, ET], F32, space="PSUM", tag="pa")
                    for q in range(4):
                        for a, b in NSL:
                            nc.tensor.matmul(cps[:, a:b],
                                             W4pt[:, q * 224 + m * 112:
                                                  q * 224 + (m + 1) * 112],
                                             rqs[q][:, a:b],
                                             start=(q == 0), stop=(q == 3))
                    cs = sp.tile([112, ET], BF16, tag=f"cs{m}")
                    nc.vector.tensor_copy(cs[:], cps[:])
                    tm = sp.tile([112, ET], BF16, tag=f"tm{m}")
                    nc.vector.tensor_tensor(out=tm[:], in0=cs[:], in1=aiS[0:112, :],
                                            op=ALU.mult)
                    tms.append(tm)
                fps0 = pa.tile([16, ET], F32, space="PSUM", tag="pa")
                for a, b in NSL:
                    nc.tensor.matmul(fps0[:, a:b], L20[:], tms[0][:, a:b],
                                     start=True, stop=False)
                    nc.tensor.matmul(fps0[:, a:b], L2x[:], tms[1][:, a:b],
                                     start=False, stop=True)
                fps1 = pa.tile([44, ET], F32, space="PSUM", tag="pa")
                for a, b in NSL:
                    nc.tensor.matmul(fps1[:, a:b], L21[:], tms[1][:, a:b],
                                     start=True, stop=True)
                F = sp.tile([96, ET], BF16, tag="F")
                nc.gpsimd.memset(F[:], 0.0)
                nc.vector.tensor_copy(F[0:16, :], fps0[:])
                ss = sp.tile([44, ET], BF16, tag="ss")
                nc.vector.tensor_copy(ss[:], fps1[:])
                nc.vector.tensor_tensor(out=F[32:64, :], in0=ss[0:32, :],
                                        in1=shs[0:32, :], op=ALU.mult)
                nc.vector.tensor_tensor(out=F[64:76, :], in0=ss[32:44, :],
                                        in1=shs[32:44, :], op=ALU.mult)
                # scatter
                for cc in range(TILE_CH):
                    cg = t * TILE_CH + cc
                    win = cg // W_CH
                    ftp = pw.tile([P, 96], BF16, space="PSUM", tag="ftp")
                    nc.tensor.transpose(ftp[:], F[:, cc * P:(cc + 1) * P],
                                        identb[0:96, 0:96])
                    fc = sp.tile([P, 60], BF16, tag="fc")
                    nc.vector.tensor_copy(fc[:, 0:16], ftp[:, 0:16])
                    nc.vector.tensor_copy(fc[:, 16:60], ftp[:, 32:76])
                    wc = pp.tile([P, 60], F32, space="PSUM", tag="pc")
                    nc.tensor.matmul(wc[:], oht[:, cc * P:(cc + 1) * P], fc[:],
                                     start=True, stop=True)
                    if cg % W_CH == 0:
                        win_sb = sp.tile([P, 60], F32, tag="winsb")
                        nc.vector.tensor_copy(win_sb[:], wc[:])
                    else:
                        nc.vector.tensor_tensor(out=win_sb[:], in0=win_sb[:],
                                                in1=wc[:], op=ALU.add)
                    if cg % W_CH == W_CH - 1:
                        nc.sync.dma_start(out_d[win * P:(win + 1) * P, :],
                                          win_sb[:])
    nc.compile()
    return nc


_CACHE = {}


def kernel(**inputs):
    per_core, onehotA, recip_pl, W_CH, C_TOT, E = _host_prep(inputs)
    et = np.asarray(inputs["embed_table"], np.float32)
    fW4 = np.asarray(inputs["fc_W4"], np.float32)
    consts = _build_consts(fW4)
    W4p, Sel, L2_0, L2_x, L2_1, Selsh = consts
    key = (W_CH, C_TOT)
    if key not in _CACHE:
        _CACHE[key] = _build_bass(W_CH, C_TOT, E, consts)
    nc = _CACHE[key]
    shared = dict(
        onehotA=onehotA, recip_pl=recip_pl,
        TA=(et @ np.asarray(inputs["fit_W1"], np.float32)).astype(np.float32),
        fit_W2=np.asarray(inputs["fit_W2"], np.float32),
        fit_W3=np.asarray(inputs["fit_W3"], np.float32),
        fc_W1p=(np.asarray(inputs["fc_W1"], np.float32) / 1.12),
        fc_W2p=(np.asarray(inputs["fc_W2"], np.float32) / 8.0),
        fc_W3p=(np.asarray(inputs["fc_W3"], np.float32) / 8.0),
        W4p=np.ascontiguousarray(np.transpose(W4p.reshape(4, 128, 224), (1, 0, 2)).reshape(128, 896)),
        cvec=np.tile(np.array([*(-VCENters / STEP), 1.0 / STEP, -1.0,
                               15.0 ** 0.5, 0.5 * 5.0 ** 0.5, 5.0 ** 0.5,
                               0.5 * 15.0 ** 0.5], np.float32), (P, 1)),
        Sel=np.ascontiguousarray(np.transpose(Sel, (1, 0, 2)).reshape(64, 512)),
        L2_0=L2_0, L2_x=L2_x, L2_1=L2_1, Selsh=Selsh,
    )
    import ml_dtypes
    for k in ("W4p", "Sel", "L2_0", "L2_x", "L2_1", "TA", "fit_W2", "fit_W3",
              "fc_W1p", "fc_W2p", "fc_W3p", "Selsh", "onehotA"):
        shared[k] = shared[k].astype(ml_dtypes.bfloat16)
    in_maps = []
    for ci in range(N_CORES):
        m = dict(shared)
        m.update(geom_pl=per_core[ci]["geom_pl"], dst_pl=per_core[ci]["dst_pl"],
                 oh_pl=per_core[ci]["oh_pl"].astype(ml_dtypes.bfloat16),
                 idx_src=per_core[ci]["idx_src"], idx_dst=per_core[ci]["idx_dst"])
        in_maps.append(m)
    res = bass_utils.run_bass_kernel_spmd(nc, in_maps, core_ids=list(range(N_CORES)))
    out = np.concatenate([res.results[ci]["out"][:NPC] for ci in range(N_CORES)], 0)
    return out.astype(np.float32)


# revision 38
# speedup vs baseline: 1.4003x; 1.2619x over previous
"""E3Conv Trainium2 kernel: 8-core SPMD, dst-partitioned edges.

Strategy: sort edges by dst; core i owns nodes [1250i,1250(i+1)) and all edges
into them (no all-reduce needed). Per core: node-MLP replicated, bf16 gather
tables for Ai/Ai*recip, radial MLP + tensor-product restructured as one
K=512 matmul per edge tile, windowed one-hot matmul scatter-mean.
"""
import sys, os
sys.path.insert(0, "/opt/trn_rl_repo")
import numpy as np

import concourse.bass as bass
import concourse.tile as tile
from concourse import bacc, mybir
from concourse import bass_utils
from concourse.masks import make_identity

P = 128
N_NODES, N_EDGES, N_GRAPHS = 10000, 131072, 64
N_CORES, NPC, N_WIN = 8, 1250, 10
MAX_RADIUS, N_BASIS = 4.0, 10
STEP = MAX_RADIUS / (N_BASIS + 1)
VCENters = np.linspace(0.0, MAX_RADIUS, N_BASIS + 2)[1:-1].astype(np.float32)
F32, BF16, F32R, I16 = (mybir.dt.float32, mybir.dt.bfloat16,
                        mybir.dt.float32r, mybir.dt.int16)
AF = mybir.ActivationFunctionType
ALU = mybir.AluOpType
NCH = 79  # node chunks of 128 (79*128 = 10112 >= 10000)


def _build_consts(fW4):
    s3 = 3.0 ** 0.5
    W4p = np.zeros((512, 224), np.float32)
    offs = {0: 0, 1: 1024, 2: 1536}
    wbase = {0: 0, 1: 16, 2: 24}
    scale_l = {0: 1.0 / 64, 1: s3 / 64, 2: 1.0 / 64}
    for l, mo in enumerate((16, 8, 4)):
        for u in range(8):
            for v in range(8):
                for wl in range(mo):
                    col = offs[l] + (u * 8 + v) * mo + wl
                    w = wbase[l] + wl
                    W4p[np.arange(64) * 8 + v, w * 8 + u] = fW4[:, col] * scale_l[l]
    Sel = np.zeros((4, 64, 128), np.float32)
    for q in range(4):
        for r in range(128):
            Sel[q, 16 * q + r // 8, r] = 1.0
    L2_0 = np.zeros((112, 16), np.float32)
    for r in range(112):
        L2_0[r, r // 8] = 1.0
    L2_x = np.zeros((112, 16), np.float32)
    for r in range(16):
        L2_x[r, 14 + r // 8] = 1.0
    L2_1 = np.zeros((112, 44), np.float32)
    for r in range(112):
        w = 14 + r // 8
        if w < 16:
            pass
        elif w < 24:
            for m in range(3):
                L2_1[r, (w - 16) * 3 + m] = 1.0
        else:
            for m in range(5):
                L2_1[r, 24 + (w - 24) * 5 + m] = 1.0
    Selsh = np.zeros((8, 44), np.float32)
    for w in range(8):
        for m in range(3):
            Selsh[m, w * 3 + m] = 1.0
    for w in range(4):
        for m in range(5):
            Selsh[3 + m, 24 + w * 5 + m] = 1.0
    return W4p, Sel, L2_0, L2_x, L2_1, Selsh


def _host_prep(inputs):
    pos = np.asarray(inputs["pos"], np.float32)
    A = np.asarray(inputs["A"]).astype(np.int64)
    batch = np.asarray(inputs["batch"]).astype(np.int64)
    esrc = np.asarray(inputs["edge_src"]).astype(np.int64)
    edst = np.asarray(inputs["edge_dst"]).astype(np.int64)
    shifts = np.asarray(inputs["edge_shifts"], np.float32)
    cell = np.asarray(inputs["cell"], np.float32)
    counts = np.bincount(edst, minlength=N_NODES).astype(np.float32)
    recipc = 1.0 / np.maximum(counts, 1.0)
    cpn = cell[batch].reshape(N_NODES, 9)
    order = np.argsort(edst, kind="stable")
    wins_all, W_CH = [], 0
    for ci in range(N_CORES):
        lo = ci * NPC
        m = order[(edst[order] >= lo) & (edst[order] < lo + NPC)]
        wins = []
        for w in range(N_WIN):
            wlo = lo + w * P
            whi = min(lo + (w + 1) * P, lo + NPC)
            wm = m[(edst[m] >= wlo) & (edst[m] < whi)]
            wins.append(wm)
            W_CH = max(W_CH, (len(wm) + P - 1) // P)
    # uniform even chunk count per window across all cores
        wins_all.append(wins)
    if W_CH % 2:
        W_CH += 1
    C_TOT = N_WIN * W_CH
    E = C_TOT * P
    onehotA = np.zeros((10, N_NODES), np.float32)
    onehotA[A, np.arange(N_NODES)] = 1.0
    recip_pl = np.ones((P, NCH), np.float32)
    rc = np.concatenate([recipc, np.ones(NCH * P - N_NODES, np.float32)])
    recip_pl[:, :] = rc.reshape(NCH, P).T
    per_core = []
    for ci in range(N_CORES):
        idx = np.zeros(E, np.int64)
        valid = np.zeros(E, bool)
        dstloc = np.full(E, -1.0, np.float32)
        for w in range(N_WIN):
            wm = wins_all[ci][w]
            s = w * W_CH * P
            idx[s:s + len(wm)] = wm
            valid[s:s + len(wm)] = True
            dstloc[s:s + len(wm)] = (edst[wm] - ci * NPC - w * P).astype(np.float32)
        src = np.where(valid, esrc[idx], 0)
        dst = np.where(valid, edst[idx], 0)
        sh = np.where(valid[:, None], shifts[idx], np.float32(1.0))
        geom = np.concatenate([pos[src], pos[dst], sh, cpn[src]], 1)  # [E,18]
        geom_pl = np.ascontiguousarray(
            np.transpose(geom.reshape(C_TOT, P, 18), (1, 2, 0)).reshape(P, 18 * C_TOT))
        dst_pl = np.ascontiguousarray(dstloc.reshape(C_TOT, P).T)

        def wrap(ix):
            wr = ix.astype(np.int16).reshape(-1, 16).T  # [16, E/16]
            return np.ascontiguousarray(np.tile(wr, (8, 1)))
        ohm = (dstloc.reshape(C_TOT, P, 1) ==
               np.arange(P, dtype=np.float32)[None, None, :])
        oh_pl = np.ascontiguousarray(
            np.transpose(ohm, (1, 0, 2)).reshape(P, C_TOT * P))
        per_core.append(dict(geom_pl=geom_pl, dst_pl=dst_pl, oh_pl=oh_pl,
                             idx_src=wrap(src), idx_dst=wrap(dst)))
    return per_core, onehotA, recip_pl, W_CH, C_TOT, E


def _build_bass(W_CH, C_TOT, E, consts):
    W4p, Sel, L2_0, L2_x, L2_1, Selsh = [c.astype(np.float32) for c in consts[:6]]
    TILE_CH = W_CH // 2
    NT = C_TOT // TILE_CH
    ET = TILE_CH * P
    NIW = E // 16
    nc = bacc.Bacc("TRN2", target_bir_lowering=False, debug=False,
                   num_devices=N_CORES)

    def din(name, shape, dt=F32):
        return nc.dram_tensor(name, shape, dt, kind="ExternalInput").ap()

    geom_d = din("geom_pl", [P, 18 * C_TOT])
    dst_d = din("dst_pl", [P, C_TOT])
    ohm_d = din("oh_pl", [P, C_TOT * P], BF16)
    isrc_d = din("idx_src", [P, NIW], I16)
    idst_d = din("idx_dst", [P, NIW], I16)
    ohA_d = din("onehotA", [10, N_NODES], BF16)
    rcp_d = din("recip_pl", [P, NCH])
    TA_d = din("TA", [10, 64], BF16)
    W2_d = din("fit_W2", [64, 32], BF16)
    W3_d = din("fit_W3", [32, 8], BF16)
    fW1_d = din("fc_W1p", [10, 64], BF16)
    fW2_d = din("fc_W2p", [64, 64], BF16)
    fW3_d = din("fc_W3p", [64, 64], BF16)
    W4p_d = din("W4p", [128, 4 * 224], BF16)
    Sel_d = din("Sel", [64, 4 * 128], BF16)
    L20_d = din("L2_0", [112, 16], BF16)
    L2x_d = din("L2_x", [112, 16], BF16)
    L21_d = din("L2_1", [112, 44], BF16)
    Ssh_d = din("Selsh", [8, 44], BF16)
    cv_d = din("cvec", [P, 16])
    out_d = nc.dram_tensor("out", [N_WIN * P, 60], F32, kind="ExternalOutput").ap()

    C = C_TOT
    with tile.TileContext(nc) as tc:
        with tc.tile_pool(name="const", bufs=1) as cp, \
             tc.tile_pool(name="sb", bufs=2) as sp, \
             tc.tile_pool(name="big", bufs=1) as bp, \
             tc.tile_pool(name="ps", bufs=1, space="PSUM") as pp, \
             tc.tile_pool(name="psa", bufs=2, space="PSUM") as pa, \
             tc.tile_pool(name="pswin", bufs=2, space="PSUM") as pw, \
             tc.tile_pool(name="dram", bufs=1, space="DRAM") as dp:
            ident = cp.tile([P, P], F32)
            make_identity(nc, ident[:])
            identb = cp.tile([P, P], BF16)
            nc.vector.tensor_copy(identb[:], ident[:])
            io_i = cp.tile([P, P], mybir.dt.int32)
            nc.gpsimd.iota(io_i[:], pattern=[[1, P]], base=0, channel_multiplier=0)
            io_f = cp.tile([P, P], F32)
            nc.vector.tensor_copy(io_f[:], io_i[:])

            def load_const(dram, shape, dt=F32):
                t = cp.tile(shape, dt, tag=dram.tensor.name)
                nc.sync.dma_start(t[:], dram[:])
                return t
            TA = load_const(TA_d, [10, 64], BF16)
            W2 = load_const(W2_d, [64, 32], BF16)
            W3 = load_const(W3_d, [32, 8], BF16)
            fW1 = load_const(fW1_d, [10, 64], BF16)
            fW2 = load_const(fW2_d, [64, 64], BF16)
            fW3 = load_const(fW3_d, [64, 64], BF16)
            W4pt = load_const(W4p_d, [128, 4 * 224], BF16)
            Selt = load_const(Sel_d, [64, 4 * 128], BF16)
            L20 = load_const(L20_d, [112, 16], BF16)
            L2x = load_const(L2x_d, [112, 16], BF16)
            L21 = load_const(L21_d, [112, 44], BF16)
            Ssh = load_const(Ssh_d, [8, 44], BF16)
            cv = load_const(cv_d, [P, 16])
            rcp = load_const(rcp_d, [P, NCH])
            ohA = bp.tile([10, N_NODES], BF16)
            nc.sync.dma_start(ohA[:], ohA_d[:])
            dstl = bp.tile([P, C], F32)
            nc.sync.dma_start(dstl[:], dst_d[:])
            isrc = bp.tile([P, NIW], I16)
            nc.sync.dma_start(isrc[:], isrc_d[:])
            idst = bp.tile([P, NIW], I16)
            nc.sync.dma_start(idst[:], idst_d[:])


            # ---------------- node MLP + gather tables ----------------
            Tsrc = dp.tile([NCH * P, P], BF16)
            Tdst = dp.tile([NCH * P, P], BF16)
            Ai_sb = bp.tile([8, NCH * P], F32)
            nc.gpsimd.memset(Ai_sb[:], 0.0)
            for j in range(20):
                s = j * 512
                n = min(512, N_NODES - s)
                h1p = pp.tile([64, 512], F32, space="PSUM", tag="pb")
                nc.tensor.matmul(h1p[:, :n], TA[:], ohA[:, s:s + n],
                                 start=True, stop=True)
                h1 = sp.tile([64, 512], BF16, tag="h1n")
                nc.scalar.activation(h1[:, :n], h1p[:, :n], AF.Silu)
                h2p = pp.tile([32, 512], F32, space="PSUM", tag="pb")
                nc.tensor.matmul(h2p[:, :n], W2[:], h1[:, :n],
                                 start=True, stop=True)
                h2 = sp.tile([32, 512], BF16, tag="h2n")
                nc.scalar.activation(h2[:, :n], h2p[:, :n], AF.Silu)
                aip = pa.tile([8, 512], F32, space="PSUM", tag="pa")
                nc.tensor.matmul(aip[:, :n], W3[:], h2[:, :n],
                                 start=True, stop=True)
                nc.vector.tensor_copy(Ai_sb[:, s:s + n], aip[:, :n])
            for c in range(NCH):
                s = c * P
                tp = pp.tile([P, 8], F32, space="PSUM", tag="pc")
                nc.tensor.transpose(tp[:], Ai_sb[:, s:s + P], ident[0:8, 0:8])
                f16a = sp.tile([P, 8], BF16, tag="f16a")
                nc.vector.tensor_copy(f16a[:], tp[:])
                f16b = sp.tile([P, 8], BF16, tag="f16b")
                nc.vector.tensor_tensor(
                    out=f16b[:], in0=tp[:],
                    in1=rcp[:, c:c + 1].to_broadcast([P, 8]), op=ALU.mult)
                repa = sp.tile([P, P], BF16, tag="repa")
                nc.vector.tensor_copy(
                    repa[:].rearrange("p (r v) -> p r v", v=8),
                    f16a[:].unsqueeze(1).to_broadcast([P, 16, 8]))
                repb = sp.tile([P, P], BF16, tag="repb")
                nc.vector.tensor_copy(
                    repb[:].rearrange("p (r v) -> p r v", v=8),
                    f16b[:].unsqueeze(1).to_broadcast([P, 16, 8]))
                nc.sync.dma_start(Tsrc[s:s + P, :], repa[:])
                nc.sync.dma_start(Tdst[s:s + P, :], repb[:])

            # ---------------- geometry (plane layout, whole E) ----------------
            gm = bp.tile([P, 18 * C], F32)
            nc.sync.dma_start(gm[:], geom_d[:])
            g3 = gm[:].rearrange("p (f c) -> p f c", f=18)
            tmp9 = bp.tile([P, 9 * C], F32)
            nc.vector.tensor_tensor(
                out=tmp9[:].rearrange("p (i j c) -> p i j c", i=3, j=3),
                in0=gm[:, 9 * C:18 * C].rearrange("p (i j c) -> p i j c", i=3, j=3),
                in1=gm[:, 6 * C:9 * C].rearrange("p (i c) -> p i c", i=3)
                    .unsqueeze(2).to_broadcast([P, 3, 3, C]),
                op=ALU.mult)
            sv = bp.tile([P, 3 * C], F32)
            nc.vector.tensor_tensor(out=sv[:], in0=tmp9[:, 0:3 * C],
                                    in1=tmp9[:, 3 * C:6 * C], op=ALU.add)
            nc.vector.tensor_tensor(out=sv[:], in0=sv[:],
                                    in1=tmp9[:, 6 * C:9 * C], op=ALU.add)
            ev = bp.tile([P, 3 * C], F32)
            nc.vector.tensor_tensor(out=ev[:], in0=g3[:, 3:6].rearrange("p f c -> p (f c)"),
                                    in1=g3[:, 0:3].rearrange("p f c -> p (f c)"),
                                    op=ALU.subtract)
            nc.vector.tensor_tensor(out=ev[:], in0=ev[:], in1=sv[:], op=ALU.add)
            sq = bp.tile([P, 3 * C], F32)
            nc.vector.tensor_tensor(out=sq[:], in0=ev[:], in1=ev[:], op=ALU.mult)
            ln2 = bp.tile([P, C], F32)
            nc.vector.tensor_tensor(out=ln2[:], in0=sq[:, 0:C], in1=sq[:, C:2 * C],
                                    op=ALU.add)
            nc.vector.tensor_tensor(out=ln2[:], in0=ln2[:], in1=sq[:, 2 * C:3 * C],
                                    op=ALU.add)
            ln = bp.tile([P, C], F32)
            nc.scalar.activation(ln[:], ln2[:], AF.Sqrt)
            rl = bp.tile([P, C], F32)
            nc.vector.reciprocal(rl[:], ln[:])
            u = bp.tile([P, 3 * C], F32)
            nc.vector.tensor_tensor(
                out=u[:].rearrange("p (f c) -> p f c", f=3),
                in0=ev[:].rearrange("p (f c) -> p f c", f=3),
                in1=rl[:].unsqueeze(1).to_broadcast([P, 3, C]), op=ALU.mult)
            usq = bp.tile([P, 3 * C], F32)
            nc.vector.tensor_tensor(out=usq[:], in0=u[:], in1=u[:], op=ALU.mult)
            # feature planes tile: f-major [basis10 | sh1 3 | sh2 5]
            gf = bp.tile([P, 18 * C], F32)
            dt2 = bp.tile([P, 10 * C], F32)
            s5, s15 = 5.0 ** 0.5, 15.0 ** 0.5
            for b in range(N_BASIS):
                nc.scalar.activation(dt2[:, b * C:(b + 1) * C], ln[:], AF.Square,
                                     bias=cv[:, b:b + 1],
                                     scale=cv[:, 10:11])
            nc.scalar.activation(gf[:, 0:10 * C], dt2[:], AF.Exp,
                                 scale=cv[:, 11:12])
            nc.vector.tensor_copy(gf[:, 10 * C:13 * C], u[:])
            t1 = bp.tile([P, C], F32)
            nc.scalar.mul(t1[:], u[:, 2 * C:3 * C], cv[:, 12:13])       # sqrt15*uz
            nc.vector.tensor_tensor(out=gf[:, 13 * C:14 * C], in0=u[:, 0:C],
                                    in1=t1[:], op=ALU.mult)     # m0
            nc.vector.tensor_tensor(out=gf[:, 16 * C:17 * C], in0=u[:, C:2 * C],
                                    in1=t1[:], op=ALU.mult)     # m3
            nc.scalar.mul(t1[:], u[:, 0:C], cv[:, 12:13])               # sqrt15*ux
            nc.vector.tensor_tensor(out=gf[:, 14 * C:15 * C], in0=u[:, C:2 * C],
                                    in1=t1[:], op=ALU.mult)     # m1
            t2 = bp.tile([P, C], F32)
            nc.vector.tensor_tensor(out=t2[:], in0=usq[:, 0:C],
                                    in1=usq[:, 2 * C:3 * C], op=ALU.add)
            nc.scalar.mul(t2[:], t2[:], cv[:, 13:14])
            t3 = bp.tile([P, C], F32)
            nc.scalar.mul(t3[:], usq[:, C:2 * C], cv[:, 14:15])
            nc.vector.tensor_tensor(out=gf[:, 15 * C:16 * C], in0=t3[:], in1=t2[:],
                                    op=ALU.subtract)            # m2
            nc.vector.tensor_tensor(out=t2[:], in0=usq[:, 2 * C:3 * C],
                                    in1=usq[:, 0:C], op=ALU.subtract)
            nc.scalar.mul(gf[:, 17 * C:18 * C], t2[:], cv[:, 15:16])  # m4
            gfv = gf[:].rearrange("p (f c) -> p f c", f=18)

            NSL = [(0, 512), (512, ET)] if ET > 512 else [(0, ET)]
            # ---------------- edge tiles ----------------
            win_sb = None
            for t in range(NT):
                cols = slice(t * ET, (t + 1) * ET)
                wcols = slice(t * (NIW // NT), (t + 1) * (NIW // NT))
                aiS = sp.tile([P, ET], BF16, tag="aiS")
                nc.gpsimd.dma_gather(
                    aiS[:].unsqueeze(1), Tsrc[:, :], isrc[:, wcols], ET, ET, P,
                    transpose=True)
                aiD = sp.tile([P, ET], BF16, tag="aiD")
                nc.gpsimd.dma_gather(
                    aiD[:].unsqueeze(1), Tdst[:, :], idst[:, wcols], ET, ET, P,
                    transpose=True)
                oht = sp.tile([P, ET], BF16, tag="oht")
                nc.sync.dma_start(oht[:], ohm_d[:, t * ET:(t + 1) * ET])
                h1p = pa.tile([64, ET], F32, space="PSUM", tag="pa")
                bts = []
                for cc in range(TILE_CH):
                    cg = t * TILE_CH + cc
                    btp = pp.tile([10, P], F32, space="PSUM", tag="pc")
                    nc.tensor.transpose(btp[:], gfv[:, 0:10, cg], ident[:])
                    bt = sp.tile([10, P], BF16, tag=f"bt{cc}")
                    nc.vector.tensor_copy(bt[:], btp[:])
                    btq = pp.tile([8, P], F32, space="PSUM", tag="pc")
                    nc.tensor.transpose(btq[:], gfv[:, 10:18, cg], ident[:])
                    bsh = sp.tile([8, P], BF16, tag=f"bsh{cc}")
                    nc.vector.tensor_copy(bsh[:], btq[:])
                    bts.append(bsh)
                    csl = slice(cc * P, (cc + 1) * P)
                    nc.tensor.matmul(h1p[:, csl], fW1[:], bt[:],
                                     start=True, stop=True)
                h1 = sp.tile([64, ET], BF16, tag="eh1")
                nc.scalar.activation(h1[:], h1p[:], AF.Silu)
                shp = pa.tile([44, ET], F32, space="PSUM", tag="pa")
                for cc in range(TILE_CH):
                    csl = slice(cc * P, (cc + 1) * P)
                    nc.tensor.matmul(shp[:, csl], Ssh[:], bts[cc][:],
                                     start=True, stop=True)
                shs = sp.tile([44, ET], BF16, tag="shs")
                nc.vector.tensor_copy(shs[:], shp[:])
                h2p = pa.tile([64, ET], F32, space="PSUM", tag="pa")
                for a, b in NSL:
                    nc.tensor.matmul(h2p[:, a:b], fW2[:], h1[:, a:b],
                                     start=True, stop=True)
                h2 = sp.tile([64, ET], BF16, tag="eh2")
                nc.scalar.activation(h2[:], h2p[:], AF.Silu)
                h3p = pa.tile([64, ET], F32, space="PSUM", tag="pa")
                for a, b in NSL:
                    nc.tensor.matmul(h3p[:, a:b], fW3[:], h2[:, a:b],
                                     start=True, stop=True)
                w3b = sp.tile([64, ET], BF16, tag="ew3b")
                nc.scalar.activation(w3b[:], h3p[:], AF.Silu)
                rqs = []
                for q in range(4):
                    wrp = pa.tile([P, ET], F32, space="PSUM", tag="pa")
                    for a, b in NSL:
                        nc.tensor.matmul(wrp[:, a:b],
                                         Selt[:, 128 * q:128 * (q + 1)],
                                         w3b[:, a:b], start=True, stop=True)
                    wrs = sp.tile([P, ET], BF16, tag=f"wrs{q}")
                    nc.vector.tensor_copy(wrs[:], wrp[:])
                    rq = sp.tile([P, ET], BF16, tag=f"rq{q}")
                    nc.vector.tensor_tensor(out=rq[:], in0=wrs[:], in1=aiD[:],
                                            op=ALU.mult)
                    rqs.append(rq)
                tms = []
                for m in range(2):
                    cps = pa.tile([112, ET], F32, space="PSUM", tag="pa")
                    for q in range(4):
                        for a, b in NSL:
                            nc.tensor.matmul(cps[:, a:b],
                                             W4pt[:, q * 224 + m * 112:
                                                  q * 224 + (m + 1) * 112],
                                             rqs[q][:, a:b],
                                             start=(q == 0), stop=(q == 3))
                    cs = sp.tile([112, ET], BF16, tag=f"cs{m}")
                    nc.vector.tensor_copy(cs[:], cps[:])
                    tm = sp.tile([112, ET], BF16, tag=f"tm{m}")
                    nc.vector.tensor_tensor(out=tm[:], in0=cs[:], in1=aiS[0:112, :],
                                            op=ALU.mult)
                    tms.append(tm)
                fps0 = pa.tile([16, ET], F32, space="PSUM", tag="pa")
                for a, b in NSL:
                    nc.tensor.matmul(fps0[:, a:b], L20[:], tms[0][:, a:b],
                                     start=True, stop=False)
                    nc.tensor.matmul(fps0[:, a:b], L2x[:], tms[1][:, a:b],
                                     start=False, stop=True)
                fps1 = pa.tile([44, ET], F32, space="PSUM", tag="pa")
                for a, b in NSL:
                    nc.tensor.matmul(fps1[:, a:b], L21[:], tms[1][:, a:b],
                                     start=True, stop=True)
                F = sp.tile([96, ET], BF16, tag="F")
                nc.gpsimd.memset(F[:], 0.0)
                nc.vector.tensor_copy(F[0:16, :], fps0[:])
                ss = sp.tile([44, ET], BF16, tag="ss")
                nc.vector.tensor_copy(ss[:], fps1[:])
                nc.vector.tensor_tensor(out=F[32:64, :], in0=ss[0:32, :],
                                        in1=shs[0:32, :], op=ALU.mult)
                nc.vector.tensor_tensor(out=F[64:76, :], in0=ss[32:44, :],
                                        in1=shs[32:44, :], op=ALU.mult)
                # scatter
                for cc in range(TILE_CH):
                    cg = t * TILE_CH + cc
                    win = cg // W_CH
                    ftp = pw.tile([P, 96], BF16, space="PSUM", tag="ftp")
                    nc.tensor.transpose(ftp[:], F[:, cc * P:(cc + 1) * P],
                                        identb[0:96, 0:96])
                    fc = sp.tile([P, 60], BF16, tag="fc")
                    nc.vector.tensor_copy(fc[:, 0:16], ftp[:, 0:16])
                    nc.vector.tensor_copy(fc[:, 16:60], ftp[:, 32:76])
                    wc = pa.tile([P, 60], F32, space="PSUM", tag="pa")
                    nc.tensor.matmul(wc[:], oht[:, cc * P:(cc + 1) * P], fc[:],
                                     start=True, stop=True)
                    if cg % W_CH == 0:
                        win_sb = sp.tile([P, 60], F32, tag="winsb")
                        nc.vector.tensor_copy(win_sb[:], wc[:])
                    else:
                        nc.vector.tensor_tensor(out=win_sb[:], in0=win_sb[:],
                                                in1=wc[:], op=ALU.add)
                    if cg % W_CH == W_CH - 1:
                        nc.sync.dma_start(out_d[win * P:(win + 1) * P, :],
                                          win_sb[:])
    nc.compile()
    return nc


_CACHE = {}


def kernel(**inputs):
    per_core, onehotA, recip_pl, W_CH, C_TOT, E = _host_prep(inputs)
    et = np.asarray(inputs["embed_table"], np.float32)
    fW4 = np.asarray(inputs["fc_W4"], np.float32)
    consts = _build_consts(fW4)
    W4p, Sel, L2_0, L2_x, L2_1, Selsh = consts
    key = (W_CH, C_TOT)
    if key not in _CACHE:
        _CACHE[key] = _build_bass(W_CH, C_TOT, E, consts)
    nc = _CACHE[key]
    shared = dict(
        onehotA=onehotA, recip_pl=recip_pl,
        TA=(et @ np.asarray(inputs["fit_W1"], np.float32)).astype(np.float32),
        fit_W2=np.asarray(inputs["fit_W2"], np.float32),
        fit_W3=np.asarray(inputs["fit_W3"], np.float32),
        fc_W1p=(np.asarray(inputs["fc_W1"], np.float32) / 1.12),
        fc_W2p=(np.asarray(inputs["fc_W2"], np.float32) / 8.0),
        fc_W3p=(np.asarray(inputs["fc_W3"], np.float32) / 8.0),
        W4p=np.ascontiguousarray(np.transpose(W4p.reshape(4, 128, 224), (1, 0, 2)).reshape(128, 896)),
        cvec=np.tile(np.array([*(-VCENters / STEP), 1.0 / STEP, -1.0,
                               15.0 ** 0.5, 0.5 * 5.0 ** 0.5, 5.0 ** 0.5,
                               0.5 * 15.0 ** 0.5], np.float32), (P, 1)),
        Sel=np.ascontiguousarray(np.transpose(Sel, (1, 0, 2)).reshape(64, 512)),
        L2_0=L2_0, L2_x=L2_x, L2_1=L2_1, Selsh=Selsh,
    )
    import ml_dtypes
    for k in ("W4p", "Sel", "L2_0", "L2_x", "L2_1", "TA", "fit_W2", "fit_W3",
              "fc_W1p", "fc_W2p", "fc_W3p", "Selsh", "onehotA"):
        shared[k] = shared[k].astype(ml_dtypes.bfloat16)
    in_maps = []
    for ci in range(N_CORES):
        m = dict(shared)
        m.update(geom_pl=per_core[ci]["geom_pl"], dst_pl=per_core[ci]["dst_pl"],
                 oh_pl=per_core[ci]["oh_pl"].astype(ml_dtypes.bfloat16),
                 idx_src=per_core[ci]["idx_src"], idx_dst=per_core[ci]["idx_dst"])
        in_maps.append(m)
    res = bass_utils.run_bass_kernel_spmd(nc, in_maps, core_ids=list(range(N_CORES)))
    out = np.concatenate([res.results[ci]["out"][:NPC] for ci in range(N_CORES)], 0)
    return out.astype(np.float32)
